# revision 20
# baseline (speedup 1.0000x reference)
"""Causal self-attention Trainium2 Bass kernel (fp8 DoubleRow version).

Problem: B=4, T=2048, DIM=1024, H=16 heads, head_dim=64 (fp32).
  qkv = x @ w_qkv.T ; per-head causal softmax(q k^T / 8) v ; out @ w_out.T

Sharding (8 cores): core c -> (batch b = c//2, head-group g = c%2 of 8 heads).
Each core computes a partial output y_partial = attn_out_g @ w_out[:, g]^T
for its batch; host sums the two head-group partials per batch.

Precision scheme (validated vs fp32 reference, sim rel max err 0.0115 < 2e-2):
  - QKV projection in fp8e4 (e4m3) with MatmulPerfMode.DoubleRow: each
    matmul contracts 2 k-subtiles of 128 per pass.
  - Scores in fp8 DoubleRow: q/k stored [32 part, 2 dim-half, tokens]
    per 4-head group (weight columns pre-ordered on host), K = 32 x 2.
  - PV in fp8 DoubleRow over key-tile pairs: lhsT = v [128, 2, 66]
    (col 64 of ones emits the softmax denominator row; col 65 zero pad;
    sub-stride 80 because dual-fp8 LDWEIGHTS requires step % 16 == 0).
  - exp computes exp(s/8 - 2): the -2 bias keeps e < 240 (fp8 max);
    it cancels in the normalization.
  - Queries 0:256 (few attended keys -> fp8 noise doesn't average out)
    use a bf16 path: bf16 QKV over tokens 0:256 + bf16 scores/PV.
  - Output projection in bf16.

Causal masking: a -240 "kill triangle" is accumulated into the scores
PSUM by an extra matmul (lhsT = diag(-240), rhs = 0/1 pattern); exp
then underflows to 0.  Work above the diagonal is skipped: diagonal
quads slice scores/exp to the live query range per key-subtile; the
fully-dead leading block of each odd subtile is never computed (its e
range is memset to 0 for the fp8 PV; the bf16 PV just slices it out).
"""

import contextlib
from collections import deque

import numpy as np
import ml_dtypes

import concourse.bass as bass
import concourse.mybir as mybir
import concourse.tile as tile
from concourse import bacc
from concourse.bass_utils import run_bass_kernel_spmd

B, T, DIM = 4, 2048, 1024
NUM_HEADS, HEAD_DIM = 16, 64
INNER = NUM_HEADS * HEAD_DIM
SCALE = HEAD_DIM ** -0.5

N_CORES = 8
HEADS_PER_CORE = 8
HG = HEADS_PER_CORE * HEAD_DIM  # 512 = inner slice per core
NCH = T // 512                  # 4 token chunks

F32 = mybir.dt.float32
BF16 = mybir.dt.bfloat16
F8 = mybir.dt.float8e4
DR = mybir.MatmulPerfMode.DoubleRow
EXPF = mybir.ActivationFunctionType.Exp

NEG = -240.0
EXP_BIAS = -2.0
BFQ = 256          # queries [0, BFQ) take the bf16 path


def xr(ap, pattern, **kw):
    return ap.rearrange(pattern, **kw)


def build_bass():
    nc = bacc.Bacc()
    xtbf = nc.declare_dram_parameter("xtbf", [DIM, BFQ], BF16, isOutput=False)
    xt8 = nc.declare_dram_parameter("xt8", [512, 2 * T], F8, isOutput=False)
    wqbf = nc.declare_dram_parameter("wqbf", [DIM, 3 * HG], BF16, isOutput=False)
    wq8 = nc.declare_dram_parameter("wq8", [512, 2 * 3 * HG], F8, isOutput=False)
    woutt = nc.declare_dram_parameter("woutt", [HG, DIM], BF16, isOutput=False)
    trik8 = nc.declare_dram_parameter("trik8", [128, 128], F8, isOutput=False)
    trikb = nc.declare_dram_parameter("trikb", [128, 128], BF16, isOutput=False)
    yt = nc.declare_dram_parameter("yt", [DIM, T], BF16, isOutput=True)

    with tile.TileContext(nc) as tc:
        _emit(nc, tc, xtbf, xt8, wqbf, wq8, woutt, trik8, trikb, yt)
    nc.finalize()
    return nc


def _emit(nc, tc, xtbf, xt8, wqbf, wq8, woutt, trik8, trikb, yt):
    ctx = contextlib.ExitStack()
    with ctx:
        singles = ctx.enter_context(tc.tile_pool(name="singles", bufs=1))
        xpool = ctx.enter_context(tc.tile_pool(name="xpool", bufs=2))
        epool = ctx.enter_context(tc.tile_pool(name="epool", bufs=3))
        apool = ctx.enter_context(tc.tile_pool(name="apool", bufs=2))
        spool = ctx.enter_context(tc.tile_pool(name="spool", bufs=1))
        dpool = ctx.enter_context(tc.tile_pool(name="dpool", bufs=2, space="DRAM"))
        # PSUM budget (8 banks of 2KB/partition):
        #   psq  [128,1024] bufs=2 -> 4 banks (score quads, double-buffered)
        #   psot [66,512]   bufs=3 -> 3 banks (per-head PV accumulators)
        #   psmm [128,512]  bufs=1 -> 1 bank (stage 1 + stage 3 groups)
        psq = ctx.enter_context(tc.tile_pool(name="psq", bufs=2, space="PSUM"))
        psot = ctx.enter_context(tc.tile_pool(name="psot", bufs=3, space="PSUM"))
        psmm = ctx.enter_context(tc.tile_pool(name="psmm", bufs=1, space="PSUM"))

        # ---- persistent SBUF tensors; DMA order = dependency order ----
        # interleave weight/x DMAs so the first stage-1 group can start after
        # only one (w, x) tile pair has landed
        wqb = []
        xts0 = []
        for k in range(8):
            w = singles.tile([128, 3 * HG], BF16, name=f"wqb{k}")
            nc.sync.dma_start(out=w, in_=wqbf[k * 128:(k + 1) * 128, :])
            wqb.append(w)
            xtile = xpool.tile([128, BFQ], BF16, tag=f"xb{k}", name=f"xb{k}")
            nc.sync.dma_start(out=xtile, in_=xtbf[k * 128:(k + 1) * 128, :])
            xts0.append(xtile)
        wq8s = []
        for k in range(4):
            w = singles.tile([128, 2, 3 * HG], F8, name=f"wq8{k}")
            nc.sync.dma_start(out=xr(w, "p s c -> p (s c)"),
                              in_=wq8[k * 128:(k + 1) * 128, :])
            wq8s.append(w)

        def x8_tiles(c):
            cs = slice(c * 512, (c + 1) * 512)
            x8s = []
            for k in range(4):
                t8 = xpool.tile([128, 2, 512], F8, tag=f"x8_{k}",
                                name=f"x8_{c}_{k}")
                nc.sync.dma_start(
                    out=t8,
                    in_=xr(xt8[k * 128:(k + 1) * 128, :],
                           "p (s t) -> p s t", s=2)[:, :, cs])
                x8s.append(t8)
            return x8s

        x8s0 = x8_tiles(0)

        wo = []
        for k in range(4):
            w = singles.tile([128, DIM], BF16, name=f"wo{k}")
            nc.sync.dma_start(out=w, in_=woutt[k * 128:(k + 1) * 128, :])
            wo.append(w)
        tk8 = singles.tile([128, 128], F8, name="tk8")
        nc.sync.dma_start(out=tk8, in_=trik8[:, :])
        tkb = singles.tile([128, 128], BF16, name="tkb")
        nc.sync.dma_start(out=tkb, in_=trikb[:, :])
        biasap = singles.tile([128, 1], F32, name="expbias")
        nc.gpsimd.memset(biasap, EXP_BIAS)

        # bf16 q/k for queries/keys [0, BFQ): 4 tiles [128, BFQ]
        qtb = [singles.tile([128, BFQ], BF16, name=f"qtb{m}") for m in range(4)]
        ktb = [singles.tile([128, BFQ], BF16, name=f"ktb{m}") for m in range(4)]
        # bf16 v for keys [0, BFQ): 2 token-tiles [128, 8, 65]
        vtb = [singles.tile([128, HEADS_PER_CORE, 65], BF16, name=f"vtb{t}")
               for t in range(BFQ // 128)]
        for t in range(BFQ // 128):
            nc.gpsimd.memset(vtb[t][:, :, 64:65], 1.0)
        # fp8 q for tokens [BFQ, T): [4h x 32d, 2 dim-half, T - BFQ]
        qt8 = [singles.tile([128, 2, T - BFQ], F8, name=f"qt8{g}")
               for g in range(2)]
        # fp8 k (all tokens): [128, 2, 2048]
        kt8 = [singles.tile([128, 2, T], F8, name=f"kt8{g}") for g in range(2)]
        # fp8 v: 8 ktile-pair tiles; inner stride 80 (dual-fp8 LDW needs
        # the sub-ktile step to be a multiple of 16), 66 cols used
        vt8 = [singles.tile([128, HEADS_PER_CORE, 2, 80], F8, name=f"vt8{tp}")
               for tp in range(8)]
        for tp in range(8):
            nc.gpsimd.memset(vt8[tp][:, :, :, 64:65], 1.0)
            nc.gpsimd.memset(vt8[tp][:, :, :, 65:66], 0.0)

        def stage1_ch0():
            """Chunk 0: bf16 QKV over [0,BFQ) + fp8 q/k/v as needed."""
            # bf16 q, k over tokens [0, BFQ)
            for which, dst in ((0, qtb), (1, ktb)):
                for m in range(4):
                    ps = psmm.tile([128, 512], F32, tag="qkv",
                                   name=f"pb{which}{m}")[:, 0:BFQ]
                    for k in range(8):
                        nc.tensor.matmul(
                            ps,
                            lhsT=wqb[k][:, which * HG + m * 128:
                                        which * HG + (m + 1) * 128],
                            rhs=xts0[k],
                            start=(k == 0), stop=(k == 7))
                    nc.vector.tensor_copy(dst[m], ps)
            # bf16 v over token tiles [0, BFQ) + fp8 casts of the same psum
            for i in range(BFQ // 128):
                ps = psmm.tile([128, 512], F32, tag="qkv", name=f"pbv{i}")
                for k in range(8):
                    nc.tensor.matmul(
                        ps,
                        lhsT=xts0[k][:, i * 128:(i + 1) * 128],
                        rhs=wqb[k][:, 2 * HG:3 * HG],
                        start=(k == 0), stop=(k == 7))
                ps3 = xr(ps, "p (h d) -> p h d", h=HEADS_PER_CORE)
                nc.vector.tensor_copy(vtb[i][:, :, 0:64], ps3)
                nc.vector.tensor_copy(vt8[i // 2][:, :, i % 2, 0:64], ps3)
            # fp8 k for chunk 0 (all 512 keys)
            for m in range(4):
                ps = psmm.tile([128, 512], F32, tag="qkv", name=f"p8k{m}")
                for k in range(4):
                    nc.tensor.matmul(
                        ps,
                        lhsT=wq8s[k][:, :, HG + m * 128:HG + (m + 1) * 128],
                        rhs=x8s0[k],
                        start=(k == 0), stop=(k == 3), perf_mode=DR)
                nc.vector.tensor_copy(kt8[m // 2][:, m % 2, 0:512], ps)
            # fp8 q for tokens [BFQ, 512)
            for m in range(4):
                ps = psmm.tile([128, 512], F32, tag="qkv",
                               name=f"p8q{m}")[:, 0:512 - BFQ]
                for k in range(4):
                    nc.tensor.matmul(
                        ps,
                        lhsT=wq8s[k][:, :, m * 128:(m + 1) * 128],
                        rhs=x8s0[k][:, :, BFQ:512],
                        start=(k == 0), stop=(k == 3), perf_mode=DR)
                nc.vector.tensor_copy(qt8[m // 2][:, m % 2, 0:512 - BFQ], ps)
            # fp8 v for token tiles [BFQ, 512)
            for i in range(BFQ // 128, 4):
                ps = psmm.tile([128, 512], F32, tag="qkv", name=f"p8v{i}")
                for k in range(4):
                    nc.tensor.matmul(
                        ps,
                        lhsT=x8s0[k][:, :, i * 128:(i + 1) * 128],
                        rhs=wq8s[k][:, :, 2 * HG:3 * HG],
                        start=(k == 0), stop=(k == 3), perf_mode=DR)
                nc.vector.tensor_copy(
                    vt8[i // 2][:, :, i % 2, 0:64],
                    xr(ps, "p (h d) -> p h d", h=HEADS_PER_CORE))

        def stage1_fp8_fillers(c):
            """fp8 QKV for chunk c>=1, as filler closures (4 matmuls each)."""
            cs = slice(c * 512, (c + 1) * 512)
            qcs = slice(c * 512 - BFQ, (c + 1) * 512 - BFQ)
            x8s = x8_tiles(c)
            fillers = []

            def qk_group(which, m):
                def go():
                    ps = psmm.tile([128, 512], F32, tag="qkv",
                                   name=f"p8{which}_{c}_{m}")
                    for k in range(4):
                        nc.tensor.matmul(
                            ps,
                            lhsT=wq8s[k][:, :, which * HG + m * 128:
                                         which * HG + (m + 1) * 128],
                            rhs=x8s[k],
                            start=(k == 0), stop=(k == 3), perf_mode=DR)
                    dst = (qt8 if which == 0 else kt8)[m // 2]
                    nc.vector.tensor_copy(
                        dst[:, m % 2, qcs if which == 0 else cs], ps)
                return go

            def v_group(i):
                def go():
                    t = 4 * c + i
                    ps = psmm.tile([128, 512], F32, tag="qkv", name=f"p8V{c}{i}")
                    for k in range(4):
                        nc.tensor.matmul(
                            ps,
                            lhsT=x8s[k][:, :, i * 128:(i + 1) * 128],
                            rhs=wq8s[k][:, :, 2 * HG:3 * HG],
                            start=(k == 0), stop=(k == 3), perf_mode=DR)
                    nc.vector.tensor_copy(
                        vt8[t // 2][:, :, t % 2, 0:64],
                        xr(ps, "p (h d) -> p h d", h=HEADS_PER_CORE))
                return go

            for m in range(4):
                fillers.append(qk_group(1, m))
            for i in range(4):
                fillers.append(v_group(i))
            for m in range(4):
                fillers.append(qk_group(0, m))
            return fillers

        # ---------------- attention building blocks ----------------
        # A quad holds scores for (head h, key-tiles 2tp and 2tp+1) over a
        # query window of width U: quad cols [sub*U, sub*U + U).
        # "live0" = first live query col (relative to the window) of sub0;
        # sub1's live range starts 128 later.  Window col 0 = query qlo.

        def scores_f8(h, quad, et, tp, qlo, U, diag):
            g, hi = h // 4, 32 * (h % 4)
            qg = qt8[g]
            qoff = qlo - BFQ
            for sub in range(2):
                t = 2 * tp + sub
                lhsT = kt8[g][hi:hi + 32, :, t * 128:(t + 1) * 128]
                base = sub * U
                if not diag:
                    nc.tensor.matmul(
                        quad[:, base:base + U],
                        lhsT=lhsT, rhs=qg[hi:hi + 32, :, qoff:qoff + U],
                        start=True, stop=True, tile_position=(hi, 0),
                        perf_mode=DR)
                    continue
                lo = sub * 128          # dead cols [0, lo) for sub1
                if sub == 1:
                    nc.gpsimd.memset(et[:, U:U + 128], 0.0)
                nc.tensor.matmul(
                    quad[:, base + lo:base + U],
                    lhsT=lhsT, rhs=qg[hi:hi + 32, :, qoff + lo:qoff + U],
                    start=True, stop=True, tile_position=(hi, 0), perf_mode=DR)

        def scores_bf(h, quad, tp, U, diag):
            m, ho = h // 2, 64 * (h % 2)
            for sub in range(2):
                t = 2 * tp + sub
                lhsT = ktb[m][ho:ho + 64, t * 128:(t + 1) * 128]
                base = sub * U
                lo = sub * 128 if diag else 0
                nc.tensor.matmul(
                    quad[:, base + lo:base + U],
                    lhsT=lhsT, rhs=qtb[m][ho:ho + 64, lo:U],
                    start=True, stop=True, tile_position=(ho, 0))

        def exp_quad(et, quad, U, diag, split, mask):
            if diag and split:
                nc.scalar.activation(et[:, 0:U], quad[:, 0:U], EXPF,
                                     scale=float(SCALE), bias=biasap)
                nc.scalar.activation(et[:, U + 128:2 * U], quad[:, U + 128:2 * U],
                                     EXPF, scale=float(SCALE), bias=biasap)
                # boundary triangles: keep-mask on the (otherwise idle) Pool
                nc.gpsimd.tensor_mul(et[:, 0:128], et[:, 0:128], mask)
                nc.gpsimd.tensor_mul(et[:, U + 128:U + 256],
                                     et[:, U + 128:U + 256], mask)
            else:
                nc.scalar.activation(et[:, 0:2 * U], quad[:, 0:2 * U], EXPF,
                                     scale=float(SCALE), bias=biasap)

        def attention(c, fillers):
            cs = slice(c * 512, (c + 1) * 512)
            aot = [apool.tile([128, 512], BF16, tag=f"aot{k}", name=f"aot{c}_{k}")
                   for k in range(4)]
            npairs = 2 * (c + 1)
            for h in range(8):
                ot = psot.tile([66, 512], F32, tag="ot", name=f"ot{c}_{h}")
                if c == 0:
                    _attn_head_ch0(h, ot)
                else:
                    for tp in range(npairs):
                        diag = tp >= npairs - 2
                        U = 256 if (diag and tp == npairs - 1) else 512
                        u0 = 512 - U
                        quad = psq.tile([128, 1024], F32, tag="quad",
                                        name=f"q{c}_{h}_{tp}")
                        et = epool.tile([128, 1024], F8, tag="e8",
                                        name=f"e{c}_{h}_{tp}")
                        scores_f8(h, quad, et, tp, c * 512 + u0, U, diag)
                        exp_quad(et, quad, U, diag, split=True, mask=tk8)
                        nc.tensor.matmul(
                            ot[:, u0:u0 + U],
                            lhsT=vt8[tp][:, h, :, 0:66],
                            rhs=xr(et[:, 0:2 * U], "p (s u) -> p s u", s=2),
                            start=(tp == 0), stop=(tp == npairs - 1),
                            perf_mode=DR)
                # normalize: row 64 of ot is the denominator
                den = spool.tile([1, 512], F32, tag="den", name=f"dn{c}_{h}")
                nc.vector.tensor_copy(den, ot[64:65, :])
                recf = spool.tile([1, 512], F32, tag="recf", name=f"rf{c}_{h}")
                nc.vector.reciprocal_approx_fast(recf, den)
                recb = spool.tile([1, 512], BF16, tag="recb", name=f"rb{c}_{h}")
                nc.vector.tensor_copy(recb, recf)
                dr = dpool.tile([1, 512], BF16, tag="dr", name=f"dr{c}_{h}")
                nc.sync.dma_start(out=dr, in_=recb)
                bcs = spool.tile([64, 512], BF16, tag="bcs", bufs=2,
                                 name=f"bs{c}_{h}")
                nc.sync.dma_start(out=bcs, in_=dr.to_broadcast((64, 512)))
                nc.vector.tensor_mul(
                    aot[h // 2][(h % 2) * 64:(h % 2) * 64 + 64, :],
                    ot[0:64, :], bcs)
                # pump fillers so the PE queue has QKV/out-proj work while
                # the exp + normalize chains of this head drain
                for _ in range(3):
                    if fillers:
                        fillers.popleft()()
            while fillers:
                fillers.popleft()()
            return aot

        def _attn_head_ch0(h, ot):
            # block A: queries [0, BFQ) in bf16 over key-tiles 0..BFQ/128-1
            UA = BFQ // 2  # window width per sub (2 ktiles span BFQ keys)
            quad = psq.tile([128, 1024], F32, tag="quad", name=f"qA_{h}")
            etA = epool.tile([128, 1024], BF16, tag="eb", name=f"eA_{h}")
            scores_bf(h, quad, 0, BFQ, diag=True)
            exp_quad(etA, quad, BFQ, diag=True, split=True, mask=tkb)
            nc.tensor.matmul(
                ot[0:65, 0:BFQ], lhsT=vtb[0][:, h, :], rhs=etA[:, 0:BFQ],
                start=True, stop=False, skip_group_check=True)
            nc.tensor.matmul(
                ot[0:65, 128:BFQ], lhsT=vtb[1][:, h, :],
                rhs=etA[:, BFQ + 128:2 * BFQ],
                start=False, stop=False, skip_group_check=True)
            # block B: queries [BFQ, 512) in fp8 over all 4 chunk-0 ktiles
            UB = 512 - BFQ
            for tp in range(2):
                diag = tp == 1  # ktiles 2,3 overlap the B query range
                quad = psq.tile([128, 1024], F32, tag="quad", name=f"qB_{h}_{tp}")
                et = epool.tile([128, 1024], F8, tag="e8", name=f"eB_{h}_{tp}")
                scores_f8(h, quad, et, tp, BFQ, UB, diag)
                exp_quad(et, quad, UB, diag, split=True, mask=tk8)
                nc.tensor.matmul(
                    ot[:, BFQ:512],
                    lhsT=vt8[tp][:, h, :, 0:66],
                    rhs=xr(et[:, 0:2 * UB], "p (s u) -> p s u", s=2),
                    start=False, stop=(tp == 1), skip_group_check=True,
                    perf_mode=DR)

        def stage3_fillers(c, aot):
            cs = slice(c * 512, (c + 1) * 512)
            fillers = []

            def out_group(od):
                def go():
                    # for the final chunk the score-quad banks are free:
                    # alternate pools so consecutive groups double-buffer
                    if c == NCH - 1 and od % 2 == 1:
                        ps = psq.tile([128, 1024], F32, tag="quad",
                                      name=f"py{c}_{od}")[:, 0:512]
                    else:
                        ps = psmm.tile([128, 512], F32, tag="qkv",
                                       name=f"py{c}_{od}")
                    for k in range(4):
                        nc.tensor.matmul(
                            ps,
                            lhsT=wo[k][:, od * 128:(od + 1) * 128],
                            rhs=aot[k],
                            start=(k == 0), stop=(k == 3))
                    ys = spool.tile([128, 512], BF16, tag="ys", bufs=2,
                                    name=f"ys{c}_{od}")
                    if c == NCH - 1 and od >= 5:
                        nc.scalar.copy(ys, ps)   # ACT is idle at the tail
                    else:
                        nc.vector.tensor_copy(ys, ps)
                    nc.sync.dma_start(out=yt[od * 128:(od + 1) * 128, cs], in_=ys)
                return go

            for od in range(8):
                fillers.append(out_group(od))
            return fillers

        # ---- schedule ----
        stage1_ch0()
        fillers = deque()
        for c in range(NCH):
            if c + 1 < NCH:
                fillers.extend(stage1_fp8_fillers(c + 1))
            aot = attention(c, fillers)
            fillers = deque(stage3_fillers(c, aot))
        while fillers:
            fillers.popleft()()


_NC_CACHE = None


def _get_nc():
    global _NC_CACHE
    if _NC_CACHE is None:
        _NC_CACHE = build_bass()
    return _NC_CACHE


def _keep_pattern():
    k = np.arange(128)[:, None]
    q = np.arange(128)[None, :]
    return (q >= k).astype(np.float32)        # 1 = attended


def make_in_maps(x, w_qkv, w_out):
    x = np.asarray(x, dtype=np.float32)
    w_qkv = np.asarray(w_qkv, dtype=np.float32)
    w_out = np.asarray(w_out, dtype=np.float32)
    pat = _keep_pattern()
    f8 = ml_dtypes.float8_e4m3
    bf = ml_dtypes.bfloat16

    in_maps = []
    for core in range(N_CORES):
        b, g = core // 2, core % 2
        gs = slice(g * HG, (g + 1) * HG)
        wsel = np.concatenate(
            [w_qkv[0 * INNER:][gs], w_qkv[1 * INNER:][gs], w_qkv[2 * INNER:][gs]],
            axis=0)                               # [1536, 1024] bf16 order
        # fp8 weight column order: q/k in (grp, dim-half) blocks of 4h x 32d
        cols = np.empty(3 * HG, np.int64)
        j = np.arange(HG)
        m, r = j // 128, j % 128
        hh = g * 8 + (m // 2) * 4 + r // 32
        d = (m % 2) * 32 + (r % 32)
        cols[0:HG] = hh * 64 + d
        cols[HG:2 * HG] = INNER + hh * 64 + d
        cols[2 * HG:] = 2 * INNER + (g * 8 + j // 64) * 64 + (j % 64)
        wsel8 = w_qkv[cols, :]                    # [1536, 1024]
        # wq8 dram [512, 3072]: row 128k+p, col i*1536+j = wsel8[j, 256k+128i+p]
        wq8d = wsel8.T.reshape(4, 2, 128, 3 * HG).transpose(0, 2, 1, 3)
        wq8d = np.ascontiguousarray(wq8d.reshape(512, 2 * 3 * HG))
        # xt8 dram [512, 4096]: row 128k+p, col i*2048+t = x[b][t, 256k+128i+p]
        xt8d = x[b].T.reshape(4, 2, 128, T).transpose(0, 2, 1, 3)
        xt8d = np.ascontiguousarray(xt8d.reshape(512, 2 * T))
        in_maps.append({
            "xtbf": np.ascontiguousarray(x[b][0:BFQ].T).astype(bf),
            "xt8": xt8d.astype(f8),
            "wqbf": np.ascontiguousarray(wsel.T).astype(bf),
            "wq8": wq8d.astype(f8),
            "woutt": np.ascontiguousarray(w_out[:, gs].T).astype(bf),
            "trik8": pat.astype(f8),
            "trikb": pat.astype(bf),
        })
    return in_maps


def kernel(x, mask, w_qkv, w_out, **_):
    nc = _get_nc()
    in_maps = make_in_maps(x, w_qkv, w_out)
    res = run_bass_kernel_spmd(nc, in_maps, core_ids=list(range(N_CORES)))
    y = np.zeros((B, T, DIM), dtype=np.float32)
    for c in range(N_CORES):
        y[c // 2] += res.results[c]["yt"].astype(np.float32).T
    return y


# revision 21
# speedup vs baseline: 1.0097x; 1.0097x over previous
"""Causal self-attention Trainium2 Bass kernel (fp8 DoubleRow version).

Problem: B=4, T=2048, DIM=1024, H=16 heads, head_dim=64 (fp32).
  qkv = x @ w_qkv.T ; per-head causal softmax(q k^T / 8) v ; out @ w_out.T

Sharding (8 cores): core c -> (batch b = c//2, head-group g = c%2 of 8 heads).
Each core computes a partial output y_partial = attn_out_g @ w_out[:, g]^T
for its batch; host sums the two head-group partials per batch.

Precision scheme (validated vs fp32 reference, sim rel max err 0.0115 < 2e-2):
  - QKV projection in fp8e4 (e4m3) with MatmulPerfMode.DoubleRow: each
    matmul contracts 2 k-subtiles of 128 per pass.
  - Scores in fp8 DoubleRow: q/k stored [32 part, 2 dim-half, tokens]
    per 4-head group (weight columns pre-ordered on host), K = 32 x 2.
  - PV in fp8 DoubleRow over key-tile pairs: lhsT = v [128, 2, 66]
    (col 64 of ones emits the softmax denominator row; col 65 zero pad;
    sub-stride 80 because dual-fp8 LDWEIGHTS requires step % 16 == 0).
  - exp computes exp(s/8 - 2): the -2 bias keeps e < 240 (fp8 max);
    it cancels in the normalization.
  - Queries 0:256 (few attended keys -> fp8 noise doesn't average out)
    use a bf16 path: bf16 QKV over tokens 0:256 + bf16 scores/PV.
  - Output projection in bf16.

Causal masking: a -240 "kill triangle" is accumulated into the scores
PSUM by an extra matmul (lhsT = diag(-240), rhs = 0/1 pattern); exp
then underflows to 0.  Work above the diagonal is skipped: diagonal
quads slice scores/exp to the live query range per key-subtile; the
fully-dead leading block of each odd subtile is never computed (its e
range is memset to 0 for the fp8 PV; the bf16 PV just slices it out).
"""

import contextlib
from collections import deque

import numpy as np
import ml_dtypes

import concourse.bass as bass
import concourse.mybir as mybir
import concourse.tile as tile
from concourse import bacc
from concourse.bass_utils import run_bass_kernel_spmd

B, T, DIM = 4, 2048, 1024
NUM_HEADS, HEAD_DIM = 16, 64
INNER = NUM_HEADS * HEAD_DIM
SCALE = HEAD_DIM ** -0.5

N_CORES = 8
HEADS_PER_CORE = 8
HG = HEADS_PER_CORE * HEAD_DIM  # 512 = inner slice per core
NCH = T // 512                  # 4 token chunks

F32 = mybir.dt.float32
BF16 = mybir.dt.bfloat16
F8 = mybir.dt.float8e4
DR = mybir.MatmulPerfMode.DoubleRow
EXPF = mybir.ActivationFunctionType.Exp

NEG = -240.0
EXP_BIAS = -2.0
BFQ = 256          # queries [0, BFQ) take the bf16 path


def xr(ap, pattern, **kw):
    return ap.rearrange(pattern, **kw)


def build_bass():
    nc = bacc.Bacc()
    xtbf = nc.declare_dram_parameter("xtbf", [DIM, BFQ], BF16, isOutput=False)
    xt8 = nc.declare_dram_parameter("xt8", [512, 2 * T], F8, isOutput=False)
    wqbf = nc.declare_dram_parameter("wqbf", [DIM, 3 * HG], BF16, isOutput=False)
    wq8 = nc.declare_dram_parameter("wq8", [512, 2 * 3 * HG], F8, isOutput=False)
    woutt = nc.declare_dram_parameter("woutt", [HG, DIM], BF16, isOutput=False)
    trik8 = nc.declare_dram_parameter("trik8", [128, 128], F8, isOutput=False)
    trikb = nc.declare_dram_parameter("trikb", [128, 128], BF16, isOutput=False)
    yt = nc.declare_dram_parameter("yt", [DIM, T], BF16, isOutput=True)

    with tile.TileContext(nc) as tc:
        _emit(nc, tc, xtbf, xt8, wqbf, wq8, woutt, trik8, trikb, yt)
    nc.finalize()
    return nc


def _emit(nc, tc, xtbf, xt8, wqbf, wq8, woutt, trik8, trikb, yt):
    ctx = contextlib.ExitStack()
    with ctx:
        singles = ctx.enter_context(tc.tile_pool(name="singles", bufs=1))
        xpool = ctx.enter_context(tc.tile_pool(name="xpool", bufs=2))
        epool = ctx.enter_context(tc.tile_pool(name="epool", bufs=3))
        apool = ctx.enter_context(tc.tile_pool(name="apool", bufs=3))
        spool = ctx.enter_context(tc.tile_pool(name="spool", bufs=1))
        dpool = ctx.enter_context(tc.tile_pool(name="dpool", bufs=2, space="DRAM"))
        # PSUM budget (8 banks of 2KB/partition):
        #   psq  [128,1024] bufs=2 -> 4 banks (score quads, double-buffered)
        #   psot [66,512]   bufs=3 -> 3 banks (per-head PV accumulators)
        #   psmm [128,512]  bufs=1 -> 1 bank (stage 1 + stage 3 groups)
        psq = ctx.enter_context(tc.tile_pool(name="psq", bufs=2, space="PSUM"))
        psot = ctx.enter_context(tc.tile_pool(name="psot", bufs=3, space="PSUM"))
        psmm = ctx.enter_context(tc.tile_pool(name="psmm", bufs=1, space="PSUM"))

        # ---- persistent SBUF tensors; DMA order = dependency order ----
        # interleave weight/x DMAs so the first stage-1 group can start after
        # only one (w, x) tile pair has landed
        wqb = []
        xts0 = []
        for k in range(8):
            w = singles.tile([128, 3 * HG], BF16, name=f"wqb{k}")
            nc.sync.dma_start(out=w, in_=wqbf[k * 128:(k + 1) * 128, :])
            wqb.append(w)
            xtile = xpool.tile([128, BFQ], BF16, tag=f"xb{k}", name=f"xb{k}")
            nc.sync.dma_start(out=xtile, in_=xtbf[k * 128:(k + 1) * 128, :])
            xts0.append(xtile)
        wq8s = []
        for k in range(4):
            w = singles.tile([128, 2, 3 * HG], F8, name=f"wq8{k}")
            nc.sync.dma_start(out=xr(w, "p s c -> p (s c)"),
                              in_=wq8[k * 128:(k + 1) * 128, :])
            wq8s.append(w)

        def x8_tiles(c):
            cs = slice(c * 512, (c + 1) * 512)
            x8s = []
            for k in range(4):
                t8 = xpool.tile([128, 2, 512], F8, tag=f"x8_{k}",
                                name=f"x8_{c}_{k}")
                nc.sync.dma_start(
                    out=t8,
                    in_=xr(xt8[k * 128:(k + 1) * 128, :],
                           "p (s t) -> p s t", s=2)[:, :, cs])
                x8s.append(t8)
            return x8s

        x8s0 = x8_tiles(0)

        wo = []
        for k in range(4):
            w = singles.tile([128, DIM], BF16, name=f"wo{k}")
            nc.sync.dma_start(out=w, in_=woutt[k * 128:(k + 1) * 128, :])
            wo.append(w)
        tk8 = singles.tile([128, 128], F8, name="tk8")
        nc.sync.dma_start(out=tk8, in_=trik8[:, :])
        tkb = singles.tile([128, 128], BF16, name="tkb")
        nc.sync.dma_start(out=tkb, in_=trikb[:, :])
        biasap = singles.tile([128, 1], F32, name="expbias")
        nc.gpsimd.memset(biasap, EXP_BIAS)

        # bf16 q/k for queries/keys [0, BFQ): 4 tiles [128, BFQ]
        qtb = [singles.tile([128, BFQ], BF16, name=f"qtb{m}") for m in range(4)]
        ktb = [singles.tile([128, BFQ], BF16, name=f"ktb{m}") for m in range(4)]
        # bf16 v for keys [0, BFQ): 2 token-tiles [128, 8, 65]
        vtb = [singles.tile([128, HEADS_PER_CORE, 65], BF16, name=f"vtb{t}")
               for t in range(BFQ // 128)]
        for t in range(BFQ // 128):
            nc.gpsimd.memset(vtb[t][:, :, 64:65], 1.0)
        # fp8 q for tokens [BFQ, T): [4h x 32d, 2 dim-half, T - BFQ]
        qt8 = [singles.tile([128, 2, T - BFQ], F8, name=f"qt8{g}")
               for g in range(2)]
        # fp8 k (all tokens): [128, 2, 2048]
        kt8 = [singles.tile([128, 2, T], F8, name=f"kt8{g}") for g in range(2)]
        # fp8 v: 8 ktile-pair tiles; inner stride 80 (dual-fp8 LDW needs
        # the sub-ktile step to be a multiple of 16), 66 cols used
        vt8 = [singles.tile([128, HEADS_PER_CORE, 2, 80], F8, name=f"vt8{tp}")
               for tp in range(8)]
        for tp in range(8):
            nc.gpsimd.memset(vt8[tp][:, :, :, 64:65], 1.0)
            nc.gpsimd.memset(vt8[tp][:, :, :, 65:66], 0.0)

        def stage1_ch0():
            """Chunk 0: bf16 QKV over [0,BFQ) + fp8 q/k/v as needed."""
            # bf16 q, k over tokens [0, BFQ)
            for which, dst in ((0, qtb), (1, ktb)):
                for m in range(4):
                    ps = psmm.tile([128, 512], F32, tag="qkv",
                                   name=f"pb{which}{m}")[:, 0:BFQ]
                    for k in range(8):
                        nc.tensor.matmul(
                            ps,
                            lhsT=wqb[k][:, which * HG + m * 128:
                                        which * HG + (m + 1) * 128],
                            rhs=xts0[k],
                            start=(k == 0), stop=(k == 7))
                    nc.vector.tensor_copy(dst[m], ps)
            # bf16 v over token tiles [0, BFQ) + fp8 casts of the same psum
            for i in range(BFQ // 128):
                ps = psmm.tile([128, 512], F32, tag="qkv", name=f"pbv{i}")
                for k in range(8):
                    nc.tensor.matmul(
                        ps,
                        lhsT=xts0[k][:, i * 128:(i + 1) * 128],
                        rhs=wqb[k][:, 2 * HG:3 * HG],
                        start=(k == 0), stop=(k == 7))
                ps3 = xr(ps, "p (h d) -> p h d", h=HEADS_PER_CORE)
                nc.vector.tensor_copy(vtb[i][:, :, 0:64], ps3)
                nc.vector.tensor_copy(vt8[i // 2][:, :, i % 2, 0:64], ps3)
            # fp8 k for chunk 0 (all 512 keys)
            for m in range(4):
                ps = psmm.tile([128, 512], F32, tag="qkv", name=f"p8k{m}")
                for k in range(4):
                    nc.tensor.matmul(
                        ps,
                        lhsT=wq8s[k][:, :, HG + m * 128:HG + (m + 1) * 128],
                        rhs=x8s0[k],
                        start=(k == 0), stop=(k == 3), perf_mode=DR)
                nc.vector.tensor_copy(kt8[m // 2][:, m % 2, 0:512], ps)
            # fp8 q for tokens [BFQ, 512)
            for m in range(4):
                ps = psmm.tile([128, 512], F32, tag="qkv",
                               name=f"p8q{m}")[:, 0:512 - BFQ]
                for k in range(4):
                    nc.tensor.matmul(
                        ps,
                        lhsT=wq8s[k][:, :, m * 128:(m + 1) * 128],
                        rhs=x8s0[k][:, :, BFQ:512],
                        start=(k == 0), stop=(k == 3), perf_mode=DR)
                nc.vector.tensor_copy(qt8[m // 2][:, m % 2, 0:512 - BFQ], ps)
            # fp8 v for token tiles [BFQ, 512)
            for i in range(BFQ // 128, 4):
                ps = psmm.tile([128, 512], F32, tag="qkv", name=f"p8v{i}")
                for k in range(4):
                    nc.tensor.matmul(
                        ps,
                        lhsT=x8s0[k][:, :, i * 128:(i + 1) * 128],
                        rhs=wq8s[k][:, :, 2 * HG:3 * HG],
                        start=(k == 0), stop=(k == 3), perf_mode=DR)
                nc.vector.tensor_copy(
                    vt8[i // 2][:, :, i % 2, 0:64],
                    xr(ps, "p (h d) -> p h d", h=HEADS_PER_CORE))

        def stage1_fp8_fillers(c):
            """fp8 QKV for chunk c>=1, as filler closures (4 matmuls each)."""
            cs = slice(c * 512, (c + 1) * 512)
            qcs = slice(c * 512 - BFQ, (c + 1) * 512 - BFQ)
            x8s = x8_tiles(c)
            fillers = []

            def qk_group(which, m):
                def go():
                    ps = psmm.tile([128, 512], F32, tag="qkv",
                                   name=f"p8{which}_{c}_{m}")
                    for k in range(4):
                        nc.tensor.matmul(
                            ps,
                            lhsT=wq8s[k][:, :, which * HG + m * 128:
                                         which * HG + (m + 1) * 128],
                            rhs=x8s[k],
                            start=(k == 0), stop=(k == 3), perf_mode=DR)
                    dst = (qt8 if which == 0 else kt8)[m // 2]
                    nc.vector.tensor_copy(
                        dst[:, m % 2, qcs if which == 0 else cs], ps)
                return go

            def v_group(i):
                def go():
                    t = 4 * c + i
                    ps = psmm.tile([128, 512], F32, tag="qkv", name=f"p8V{c}{i}")
                    for k in range(4):
                        nc.tensor.matmul(
                            ps,
                            lhsT=x8s[k][:, :, i * 128:(i + 1) * 128],
                            rhs=wq8s[k][:, :, 2 * HG:3 * HG],
                            start=(k == 0), stop=(k == 3), perf_mode=DR)
                    nc.vector.tensor_copy(
                        vt8[t // 2][:, :, t % 2, 0:64],
                        xr(ps, "p (h d) -> p h d", h=HEADS_PER_CORE))
                return go

            for m in range(4):
                fillers.append(qk_group(1, m))
            for i in range(4):
                fillers.append(v_group(i))
            for m in range(4):
                fillers.append(qk_group(0, m))
            return fillers

        # ---------------- attention building blocks ----------------
        # A quad holds scores for (head h, key-tiles 2tp and 2tp+1) over a
        # query window of width U: quad cols [sub*U, sub*U + U).
        # "live0" = first live query col (relative to the window) of sub0;
        # sub1's live range starts 128 later.  Window col 0 = query qlo.

        def scores_f8(h, quad, et, tp, qlo, U, diag):
            g, hi = h // 4, 32 * (h % 4)
            qg = qt8[g]
            qoff = qlo - BFQ
            for sub in range(2):
                t = 2 * tp + sub
                lhsT = kt8[g][hi:hi + 32, :, t * 128:(t + 1) * 128]
                base = sub * U
                if not diag:
                    nc.tensor.matmul(
                        quad[:, base:base + U],
                        lhsT=lhsT, rhs=qg[hi:hi + 32, :, qoff:qoff + U],
                        start=True, stop=True, tile_position=(hi, 0),
                        perf_mode=DR)
                    continue
                lo = sub * 128          # dead cols [0, lo) for sub1
                if sub == 1:
                    nc.gpsimd.memset(et[:, U:U + 128], 0.0)
                nc.tensor.matmul(
                    quad[:, base + lo:base + U],
                    lhsT=lhsT, rhs=qg[hi:hi + 32, :, qoff + lo:qoff + U],
                    start=True, stop=True, tile_position=(hi, 0), perf_mode=DR)

        def scores_bf(h, quad, tp, U, diag):
            m, ho = h // 2, 64 * (h % 2)
            for sub in range(2):
                t = 2 * tp + sub
                lhsT = ktb[m][ho:ho + 64, t * 128:(t + 1) * 128]
                base = sub * U
                lo = sub * 128 if diag else 0
                nc.tensor.matmul(
                    quad[:, base + lo:base + U],
                    lhsT=lhsT, rhs=qtb[m][ho:ho + 64, lo:U],
                    start=True, stop=True, tile_position=(ho, 0))

        def exp_quad(et, quad, U, diag, split, mask):
            if diag and split:
                nc.scalar.activation(et[:, 0:U], quad[:, 0:U], EXPF,
                                     scale=float(SCALE), bias=biasap)
                nc.scalar.activation(et[:, U + 128:2 * U], quad[:, U + 128:2 * U],
                                     EXPF, scale=float(SCALE), bias=biasap)
                # boundary triangles: keep-mask on the (otherwise idle) Pool
                nc.gpsimd.tensor_mul(et[:, 0:128], et[:, 0:128], mask)
                nc.gpsimd.tensor_mul(et[:, U + 128:U + 256],
                                     et[:, U + 128:U + 256], mask)
            else:
                nc.scalar.activation(et[:, 0:2 * U], quad[:, 0:2 * U], EXPF,
                                     scale=float(SCALE), bias=biasap)

        def attention(c, fillers):
            cs = slice(c * 512, (c + 1) * 512)
            aot = [apool.tile([128, 512], BF16, tag=f"aot{k}", name=f"aot{c}_{k}")
                   for k in range(4)]
            npairs = 2 * (c + 1)
            for h in range(8):
                ot = psot.tile([66, 512], F32, tag="ot", name=f"ot{c}_{h}")
                if c == 0:
                    _attn_head_ch0(h, ot)
                else:
                    for tp in range(npairs):
                        diag = tp >= npairs - 2
                        U = 256 if (diag and tp == npairs - 1) else 512
                        u0 = 512 - U
                        quad = psq.tile([128, 1024], F32, tag="quad",
                                        name=f"q{c}_{h}_{tp}")
                        et = epool.tile([128, 1024], F8, tag="e8",
                                        name=f"e{c}_{h}_{tp}")
                        scores_f8(h, quad, et, tp, c * 512 + u0, U, diag)
                        exp_quad(et, quad, U, diag, split=True, mask=tk8)
                        nc.tensor.matmul(
                            ot[:, u0:u0 + U],
                            lhsT=vt8[tp][:, h, :, 0:66],
                            rhs=xr(et[:, 0:2 * U], "p (s u) -> p s u", s=2),
                            start=(tp == 0), stop=(tp == npairs - 1),
                            perf_mode=DR)
                # normalize: row 64 of ot is the denominator
                den = spool.tile([1, 512], F32, tag="den", name=f"dn{c}_{h}")
                nc.vector.tensor_copy(den, ot[64:65, :])
                recf = spool.tile([1, 512], F32, tag="recf", name=f"rf{c}_{h}")
                nc.vector.reciprocal_approx_fast(recf, den)
                recb = spool.tile([1, 512], BF16, tag="recb", name=f"rb{c}_{h}")
                nc.vector.tensor_copy(recb, recf)
                dr = dpool.tile([1, 512], BF16, tag="dr", name=f"dr{c}_{h}")
                nc.sync.dma_start(out=dr, in_=recb)
                bcs = spool.tile([64, 512], BF16, tag="bcs", bufs=2,
                                 name=f"bs{c}_{h}")
                nc.sync.dma_start(out=bcs, in_=dr.to_broadcast((64, 512)))
                nc.vector.tensor_mul(
                    aot[h // 2][(h % 2) * 64:(h % 2) * 64 + 64, :],
                    ot[0:64, :], bcs)
                # pump fillers so the PE queue has QKV/out-proj work while
                # the exp + normalize chains of this head drain
                for _ in range(3):
                    if fillers:
                        fillers.popleft()()
            while fillers:
                fillers.popleft()()
            return aot

        def _attn_head_ch0(h, ot):
            # block A: queries [0, BFQ) in bf16 over key-tiles 0..BFQ/128-1
            UA = BFQ // 2  # window width per sub (2 ktiles span BFQ keys)
            quad = psq.tile([128, 1024], F32, tag="quad", name=f"qA_{h}")
            etA = epool.tile([128, 1024], BF16, tag="eb", name=f"eA_{h}")
            scores_bf(h, quad, 0, BFQ, diag=True)
            exp_quad(etA, quad, BFQ, diag=True, split=True, mask=tkb)
            nc.tensor.matmul(
                ot[0:65, 0:BFQ], lhsT=vtb[0][:, h, :], rhs=etA[:, 0:BFQ],
                start=True, stop=False, skip_group_check=True)
            nc.tensor.matmul(
                ot[0:65, 128:BFQ], lhsT=vtb[1][:, h, :],
                rhs=etA[:, BFQ + 128:2 * BFQ],
                start=False, stop=False, skip_group_check=True)
            # block B: queries [BFQ, 512) in fp8 over all 4 chunk-0 ktiles
            UB = 512 - BFQ
            for tp in range(2):
                diag = tp == 1  # ktiles 2,3 overlap the B query range
                quad = psq.tile([128, 1024], F32, tag="quad", name=f"qB_{h}_{tp}")
                et = epool.tile([128, 1024], F8, tag="e8", name=f"eB_{h}_{tp}")
                scores_f8(h, quad, et, tp, BFQ, UB, diag)
                exp_quad(et, quad, UB, diag, split=True, mask=tk8)
                nc.tensor.matmul(
                    ot[:, BFQ:512],
                    lhsT=vt8[tp][:, h, :, 0:66],
                    rhs=xr(et[:, 0:2 * UB], "p (s u) -> p s u", s=2),
                    start=False, stop=(tp == 1), skip_group_check=True,
                    perf_mode=DR)

        def stage3_fillers(c, aot):
            cs = slice(c * 512, (c + 1) * 512)
            fillers = []

            def out_group(od):
                def go():
                    # for the final chunk the score-quad banks are free:
                    # alternate pools so consecutive groups double-buffer
                    if c == NCH - 1 and od % 2 == 1:
                        ps = psq.tile([128, 1024], F32, tag="quad",
                                      name=f"py{c}_{od}")[:, 0:512]
                    else:
                        ps = psmm.tile([128, 512], F32, tag="qkv",
                                       name=f"py{c}_{od}")
                    for k in range(4):
                        nc.tensor.matmul(
                            ps,
                            lhsT=wo[k][:, od * 128:(od + 1) * 128],
                            rhs=aot[k],
                            start=(k == 0), stop=(k == 3))
                    ys = spool.tile([128, 512], BF16, tag="ys", bufs=2,
                                    name=f"ys{c}_{od}")
                    if c == NCH - 1 and od >= 5:
                        nc.scalar.copy(ys, ps)   # ACT is idle at the tail
                    else:
                        nc.vector.tensor_copy(ys, ps)
                    nc.sync.dma_start(out=yt[od * 128:(od + 1) * 128, cs], in_=ys)
                return go

            for od in range(8):
                fillers.append(out_group(od))
            return fillers

        # ---- schedule ----
        # stage-3 fillers are deferred a chunk so the later (bigger)
        # attention chunks keep the PE fed while exp chains drain
        stage1_ch0()
        s3 = {}
        for c in range(NCH):
            fl = deque()
            if c + 1 < NCH:
                fl.extend(stage1_fp8_fillers(c + 1))
            if c == 1:
                fl.extend(s3[0])
            elif c == 2:
                fl.extend(s3[1][:4])
            elif c == 3:
                fl.extend(s3[1][4:])
                fl.extend(s3[2])
            aot = attention(c, fl)
            s3[c] = stage3_fillers(c, aot)
        for f in s3[NCH - 1]:
            f()


_NC_CACHE = None


def _get_nc():
    global _NC_CACHE
    if _NC_CACHE is None:
        _NC_CACHE = build_bass()
    return _NC_CACHE


def _keep_pattern():
    k = np.arange(128)[:, None]
    q = np.arange(128)[None, :]
    return (q >= k).astype(np.float32)        # 1 = attended


def make_in_maps(x, w_qkv, w_out):
    x = np.asarray(x, dtype=np.float32)
    w_qkv = np.asarray(w_qkv, dtype=np.float32)
    w_out = np.asarray(w_out, dtype=np.float32)
    pat = _keep_pattern()
    f8 = ml_dtypes.float8_e4m3
    bf = ml_dtypes.bfloat16

    in_maps = []
    for core in range(N_CORES):
        b, g = core // 2, core % 2
        gs = slice(g * HG, (g + 1) * HG)
        wsel = np.concatenate(
            [w_qkv[0 * INNER:][gs], w_qkv[1 * INNER:][gs], w_qkv[2 * INNER:][gs]],
            axis=0)                               # [1536, 1024] bf16 order
        # fp8 weight column order: q/k in (grp, dim-half) blocks of 4h x 32d
        cols = np.empty(3 * HG, np.int64)
        j = np.arange(HG)
        m, r = j // 128, j % 128
        hh = g * 8 + (m // 2) * 4 + r // 32
        d = (m % 2) * 32 + (r % 32)
        cols[0:HG] = hh * 64 + d
        cols[HG:2 * HG] = INNER + hh * 64 + d
        cols[2 * HG:] = 2 * INNER + (g * 8 + j // 64) * 64 + (j % 64)
        wsel8 = w_qkv[cols, :]                    # [1536, 1024]
        # wq8 dram [512, 3072]: row 128k+p, col i*1536+j = wsel8[j, 256k+128i+p]
        wq8d = wsel8.T.reshape(4, 2, 128, 3 * HG).transpose(0, 2, 1, 3)
        wq8d = np.ascontiguousarray(wq8d.reshape(512, 2 * 3 * HG))
        # xt8 dram [512, 4096]: row 128k+p, col i*2048+t = x[b][t, 256k+128i+p]
        xt8d = x[b].T.reshape(4, 2, 128, T).transpose(0, 2, 1, 3)
        xt8d = np.ascontiguousarray(xt8d.reshape(512, 2 * T))
        in_maps.append({
            "xtbf": np.ascontiguousarray(x[b][0:BFQ].T).astype(bf),
            "xt8": xt8d.astype(f8),
            "wqbf": np.ascontiguousarray(wsel.T).astype(bf),
            "wq8": wq8d.astype(f8),
            "woutt": np.ascontiguousarray(w_out[:, gs].T).astype(bf),
            "trik8": pat.astype(f8),
            "trikb": pat.astype(bf),
        })
    return in_maps


def kernel(x, mask, w_qkv, w_out, **_):
    nc = _get_nc()
    in_maps = make_in_maps(x, w_qkv, w_out)
    res = run_bass_kernel_spmd(nc, in_maps, core_ids=list(range(N_CORES)))
    y = np.zeros((B, T, DIM), dtype=np.float32)
    for c in range(N_CORES):
        y[c // 2] += res.results[c]["yt"].astype(np.float32).T
    return y


# revision 22
# speedup vs baseline: 1.0322x; 1.0223x over previous
"""Causal self-attention Trainium2 Bass kernel (fp8 DoubleRow version).

Problem: B=4, T=2048, DIM=1024, H=16 heads, head_dim=64 (fp32).
  qkv = x @ w_qkv.T ; per-head causal softmax(q k^T / 8) v ; out @ w_out.T

Sharding (8 cores): core c -> (batch b = c//2, head-group g = c%2 of 8 heads).
Each core computes a partial output y_partial = attn_out_g @ w_out[:, g]^T
for its batch; host sums the two head-group partials per batch.

Precision scheme (validated vs fp32 reference, sim rel max err 0.0115 < 2e-2):
  - QKV projection in fp8e4 (e4m3) with MatmulPerfMode.DoubleRow: each
    matmul contracts 2 k-subtiles of 128 per pass.
  - Scores in fp8 DoubleRow: q/k stored [32 part, 2 dim-half, tokens]
    per 4-head group (weight columns pre-ordered on host), K = 32 x 2.
  - PV in fp8 DoubleRow over key-tile pairs: lhsT = v [128, 2, 66]
    (col 64 of ones emits the softmax denominator row; col 65 zero pad;
    sub-stride 80 because dual-fp8 LDWEIGHTS requires step % 16 == 0).
  - exp computes exp(s/8 - 2): the -2 bias keeps e < 240 (fp8 max);
    it cancels in the normalization.
  - Queries 0:256 (few attended keys -> fp8 noise doesn't average out)
    use a bf16 path: bf16 QKV over tokens 0:256 + bf16 scores/PV.
  - Output projection in bf16.

Causal masking: work above the diagonal is skipped.  Diagonal quads
slice scores/exp to the live query range per key-subtile; the
fully-dead leading block of each odd subtile is never computed (its e
range is memset to 0 for the fp8 PV; the bf16 PV just slices it out);
the 128x128 boundary triangle of each subtile is zeroed by a keep-mask
multiply on the (otherwise idle) GpSimd/Pool engine after exp.

The PE work is interleaved: QKV projection ("stage 1") of chunk c+1
and output projection ("stage 3") of earlier chunks are pumped as
fillers between attention heads, so the tensor engine stays busy while
exp/normalize chains drain (idle PE gaps also drop the HAM clock gate
to half rate, so continuous occupancy matters twice).
"""

import contextlib
from collections import deque

import numpy as np
import ml_dtypes

import concourse.mybir as mybir
import concourse.tile as tile
from concourse import bacc
from concourse.bass_utils import run_bass_kernel_spmd

B, T, DIM = 4, 2048, 1024
NUM_HEADS, HEAD_DIM = 16, 64
INNER = NUM_HEADS * HEAD_DIM
SCALE = HEAD_DIM ** -0.5

N_CORES = 8
HEADS_PER_CORE = 8
HG = HEADS_PER_CORE * HEAD_DIM  # 512 = inner slice per core
NCH = T // 512                  # 4 token chunks

F32 = mybir.dt.float32
BF16 = mybir.dt.bfloat16
F8 = mybir.dt.float8e4
DR = mybir.MatmulPerfMode.DoubleRow
EXPF = mybir.ActivationFunctionType.Exp

EXP_BIAS = -2.0
BFQ = 256          # queries [0, BFQ) take the bf16 path


def xr(ap, pattern, **kw):
    return ap.rearrange(pattern, **kw)


def build_bass():
    nc = bacc.Bacc()
    xtbf = nc.declare_dram_parameter("xtbf", [DIM, BFQ], BF16, isOutput=False)
    xt8 = nc.declare_dram_parameter("xt8", [512, 2 * T], F8, isOutput=False)
    wqbf = nc.declare_dram_parameter("wqbf", [DIM, 3 * HG], BF16, isOutput=False)
    wq8 = nc.declare_dram_parameter("wq8", [512, 2 * 3 * HG], F8, isOutput=False)
    woutt = nc.declare_dram_parameter("woutt", [HG, DIM], BF16, isOutput=False)
    trik8 = nc.declare_dram_parameter("trik8", [128, 128], F8, isOutput=False)
    trikb = nc.declare_dram_parameter("trikb", [128, 128], BF16, isOutput=False)
    yt = nc.declare_dram_parameter("yt", [DIM, T], BF16, isOutput=True)

    with tile.TileContext(nc) as tc:
        _emit(nc, tc, xtbf, xt8, wqbf, wq8, woutt, trik8, trikb, yt)
    nc.finalize()
    return nc


def _emit(nc, tc, xtbf, xt8, wqbf, wq8, woutt, trik8, trikb, yt):
    ctx = contextlib.ExitStack()
    with ctx:
        singles = ctx.enter_context(tc.tile_pool(name="singles", bufs=1))
        xpool = ctx.enter_context(tc.tile_pool(name="xpool", bufs=2))
        epool = ctx.enter_context(tc.tile_pool(name="epool", bufs=3))
        apool = ctx.enter_context(tc.tile_pool(name="apool", bufs=3))
        spool = ctx.enter_context(tc.tile_pool(name="spool", bufs=1))
        dpool = ctx.enter_context(tc.tile_pool(name="dpool", bufs=2, space="DRAM"))
        # PSUM budget (8 banks of 2KB/partition):
        #   psq  [128,1024] bufs=2 -> 4 banks (score quads, double-buffered)
        #   psot [66,512]   bufs=3 -> 3 banks (per-head PV accumulators)
        #   psmm [128,512]  bufs=1 -> 1 bank (stage 1 + stage 3 groups)
        psq = ctx.enter_context(tc.tile_pool(name="psq", bufs=2, space="PSUM"))
        psot = ctx.enter_context(tc.tile_pool(name="psot", bufs=3, space="PSUM"))
        psmm = ctx.enter_context(tc.tile_pool(name="psmm", bufs=1, space="PSUM"))

        # ---- persistent SBUF tensors; DMA order = dependency order ----
        # interleave weight/x DMAs so the first stage-1 group can start after
        # only one (w, x) tile pair has landed
        wqb = []
        xts0 = []
        for k in range(8):
            w = singles.tile([128, 3 * HG], BF16, name=f"wqb{k}")
            nc.sync.dma_start(out=w, in_=wqbf[k * 128:(k + 1) * 128, :])
            wqb.append(w)
            xtile = xpool.tile([128, BFQ], BF16, tag=f"xb{k}", name=f"xb{k}")
            nc.sync.dma_start(out=xtile, in_=xtbf[k * 128:(k + 1) * 128, :])
            xts0.append(xtile)
        wq8s = []
        for k in range(4):
            w = singles.tile([128, 2, 3 * HG], F8, name=f"wq8{k}")
            nc.sync.dma_start(out=xr(w, "p s c -> p (s c)"),
                              in_=wq8[k * 128:(k + 1) * 128, :])
            wq8s.append(w)

        def x8_tiles(c):
            cs = slice(c * 512, (c + 1) * 512)
            x8s = []
            for k in range(4):
                t8 = xpool.tile([128, 2, 512], F8, tag=f"x8_{k}",
                                name=f"x8_{c}_{k}")
                nc.sync.dma_start(
                    out=t8,
                    in_=xr(xt8[k * 128:(k + 1) * 128, :],
                           "p (s t) -> p s t", s=2)[:, :, cs])
                x8s.append(t8)
            return x8s

        x8s0 = x8_tiles(0)

        wo = []
        for k in range(4):
            w = singles.tile([128, DIM], BF16, name=f"wo{k}")
            nc.sync.dma_start(out=w, in_=woutt[k * 128:(k + 1) * 128, :])
            wo.append(w)
        tk8 = singles.tile([128, 128], F8, name="tk8")
        nc.sync.dma_start(out=tk8, in_=trik8[:, :])
        tkb = singles.tile([128, 128], BF16, name="tkb")
        nc.sync.dma_start(out=tkb, in_=trikb[:, :])
        biasap = singles.tile([128, 1], F32, name="expbias")
        nc.gpsimd.memset(biasap, EXP_BIAS)

        # bf16 q/k for queries/keys [0, BFQ): 4 tiles [128, BFQ]
        qtb = [singles.tile([128, BFQ], BF16, name=f"qtb{m}") for m in range(4)]
        ktb = [singles.tile([128, BFQ], BF16, name=f"ktb{m}") for m in range(4)]
        # bf16 v for keys [0, BFQ): 2 token-tiles [128, 8, 65]
        vtb = [singles.tile([128, HEADS_PER_CORE, 65], BF16, name=f"vtb{t}")
               for t in range(BFQ // 128)]
        for t in range(BFQ // 128):
            nc.gpsimd.memset(vtb[t][:, :, 64:65], 1.0)
        # fp8 q for tokens [BFQ, T): [4h x 32d, 2 dim-half, T - BFQ]
        qt8 = [singles.tile([128, 2, T - BFQ], F8, name=f"qt8{g}")
               for g in range(2)]
        # fp8 k (all tokens): [128, 2, 2048]
        kt8 = [singles.tile([128, 2, T], F8, name=f"kt8{g}") for g in range(2)]
        # fp8 v: 8 ktile-pair tiles; inner stride 80 (dual-fp8 LDW needs
        # the sub-ktile step to be a multiple of 16), 66 cols used
        vt8 = [singles.tile([128, HEADS_PER_CORE, 2, 80], F8, name=f"vt8{tp}")
               for tp in range(8)]
        for tp in range(8):
            nc.gpsimd.memset(vt8[tp][:, :, :, 64:65], 1.0)
            nc.gpsimd.memset(vt8[tp][:, :, :, 65:66], 0.0)

        def stage1_ch0():
            """Chunk 0: bf16 QKV over [0,BFQ) + fp8 q/k/v as needed."""
            # bf16 q, k over tokens [0, BFQ)
            for which, dst in ((0, qtb), (1, ktb)):
                for m in range(4):
                    ps = psmm.tile([128, 512], F32, tag="qkv",
                                   name=f"pb{which}{m}")[:, 0:BFQ]
                    for k in range(8):
                        nc.tensor.matmul(
                            ps,
                            lhsT=wqb[k][:, which * HG + m * 128:
                                        which * HG + (m + 1) * 128],
                            rhs=xts0[k],
                            start=(k == 0), stop=(k == 7))
                    nc.vector.tensor_copy(dst[m], ps)
            # bf16 v over token tiles [0, BFQ) + fp8 casts of the same psum
            for i in range(BFQ // 128):
                ps = psmm.tile([128, 512], F32, tag="qkv", name=f"pbv{i}")
                for k in range(8):
                    nc.tensor.matmul(
                        ps,
                        lhsT=xts0[k][:, i * 128:(i + 1) * 128],
                        rhs=wqb[k][:, 2 * HG:3 * HG],
                        start=(k == 0), stop=(k == 7))
                ps3 = xr(ps, "p (h d) -> p h d", h=HEADS_PER_CORE)
                nc.vector.tensor_copy(vtb[i][:, :, 0:64], ps3)
                nc.vector.tensor_copy(vt8[i // 2][:, :, i % 2, 0:64], ps3)
            # fp8 k for chunk 0 (all 512 keys)
            for m in range(4):
                ps = psmm.tile([128, 512], F32, tag="qkv", name=f"p8k{m}")
                for k in range(4):
                    nc.tensor.matmul(
                        ps,
                        lhsT=wq8s[k][:, :, HG + m * 128:HG + (m + 1) * 128],
                        rhs=x8s0[k],
                        start=(k == 0), stop=(k == 3), perf_mode=DR)
                nc.vector.tensor_copy(kt8[m // 2][:, m % 2, 0:512], ps)
            # fp8 q for tokens [BFQ, 512)
            for m in range(4):
                ps = psmm.tile([128, 512], F32, tag="qkv",
                               name=f"p8q{m}")[:, 0:512 - BFQ]
                for k in range(4):
                    nc.tensor.matmul(
                        ps,
                        lhsT=wq8s[k][:, :, m * 128:(m + 1) * 128],
                        rhs=x8s0[k][:, :, BFQ:512],
                        start=(k == 0), stop=(k == 3), perf_mode=DR)
                nc.vector.tensor_copy(qt8[m // 2][:, m % 2, 0:512 - BFQ], ps)
            # fp8 v for token tiles [BFQ, 512)
            for i in range(BFQ // 128, 4):
                ps = psmm.tile([128, 512], F32, tag="qkv", name=f"p8v{i}")
                for k in range(4):
                    nc.tensor.matmul(
                        ps,
                        lhsT=x8s0[k][:, :, i * 128:(i + 1) * 128],
                        rhs=wq8s[k][:, :, 2 * HG:3 * HG],
                        start=(k == 0), stop=(k == 3), perf_mode=DR)
                nc.vector.tensor_copy(
                    vt8[i // 2][:, :, i % 2, 0:64],
                    xr(ps, "p (h d) -> p h d", h=HEADS_PER_CORE))

        def stage1_fp8_fillers(c):
            """fp8 QKV for chunk c>=1, as filler closures (4 matmuls each)."""
            cs = slice(c * 512, (c + 1) * 512)
            qcs = slice(c * 512 - BFQ, (c + 1) * 512 - BFQ)
            x8s = x8_tiles(c)
            fillers = []

            def qk_group(which, m):
                def go():
                    ps = psmm.tile([128, 512], F32, tag="qkv",
                                   name=f"p8{which}_{c}_{m}")
                    for k in range(4):
                        nc.tensor.matmul(
                            ps,
                            lhsT=wq8s[k][:, :, which * HG + m * 128:
                                         which * HG + (m + 1) * 128],
                            rhs=x8s[k],
                            start=(k == 0), stop=(k == 3), perf_mode=DR)
                    dst = (qt8 if which == 0 else kt8)[m // 2]
                    nc.vector.tensor_copy(
                        dst[:, m % 2, qcs if which == 0 else cs], ps)
                return go

            def v_group(i):
                def go():
                    t = 4 * c + i
                    ps = psmm.tile([128, 512], F32, tag="qkv", name=f"p8V{c}{i}")
                    for k in range(4):
                        nc.tensor.matmul(
                            ps,
                            lhsT=x8s[k][:, :, i * 128:(i + 1) * 128],
                            rhs=wq8s[k][:, :, 2 * HG:3 * HG],
                            start=(k == 0), stop=(k == 3), perf_mode=DR)
                    nc.vector.tensor_copy(
                        vt8[t // 2][:, :, t % 2, 0:64],
                        xr(ps, "p (h d) -> p h d", h=HEADS_PER_CORE))
                return go

            for m in range(4):
                fillers.append(qk_group(1, m))
            for i in range(4):
                fillers.append(v_group(i))
            for m in range(4):
                fillers.append(qk_group(0, m))
            return fillers

        # ---------------- attention building blocks ----------------
        # A quad holds scores for (head h, key-tiles 2tp and 2tp+1) over a
        # query window of width U: quad cols [sub*U, sub*U + U).
        # "live0" = first live query col (relative to the window) of sub0;
        # sub1's live range starts 128 later.  Window col 0 = query qlo.

        def scores_f8(h, quad, et, tp, qlo, U, diag):
            g, hi = h // 4, 32 * (h % 4)
            qg = qt8[g]
            qoff = qlo - BFQ
            for sub in range(2):
                t = 2 * tp + sub
                lhsT = kt8[g][hi:hi + 32, :, t * 128:(t + 1) * 128]
                base = sub * U
                if not diag:
                    nc.tensor.matmul(
                        quad[:, base:base + U],
                        lhsT=lhsT, rhs=qg[hi:hi + 32, :, qoff:qoff + U],
                        start=True, stop=True, tile_position=(hi, 0),
                        perf_mode=DR)
                    continue
                lo = sub * 128          # dead cols [0, lo) for sub1
                if sub == 1:
                    nc.gpsimd.memset(et[:, U:U + 128], 0.0)
                nc.tensor.matmul(
                    quad[:, base + lo:base + U],
                    lhsT=lhsT, rhs=qg[hi:hi + 32, :, qoff + lo:qoff + U],
                    start=True, stop=True, tile_position=(hi, 0), perf_mode=DR)

        def scores_bf(h, quad, tp, U, diag):
            m, ho = h // 2, 64 * (h % 2)
            for sub in range(2):
                t = 2 * tp + sub
                lhsT = ktb[m][ho:ho + 64, t * 128:(t + 1) * 128]
                base = sub * U
                lo = sub * 128 if diag else 0
                nc.tensor.matmul(
                    quad[:, base + lo:base + U],
                    lhsT=lhsT, rhs=qtb[m][ho:ho + 64, lo:U],
                    start=True, stop=True, tile_position=(ho, 0))

        def exp_quad(et, quad, U, diag, split, mask):
            if diag and split:
                nc.scalar.activation(et[:, 0:U], quad[:, 0:U], EXPF,
                                     scale=float(SCALE), bias=biasap)
                nc.scalar.activation(et[:, U + 128:2 * U], quad[:, U + 128:2 * U],
                                     EXPF, scale=float(SCALE), bias=biasap)
                # boundary triangles: keep-mask on the (otherwise idle) Pool
                nc.gpsimd.tensor_mul(et[:, 0:128], et[:, 0:128], mask)
                nc.gpsimd.tensor_mul(et[:, U + 128:U + 256],
                                     et[:, U + 128:U + 256], mask)
            else:
                nc.scalar.activation(et[:, 0:2 * U], quad[:, 0:2 * U], EXPF,
                                     scale=float(SCALE), bias=biasap)

        def attention(c, fillers):
            cs = slice(c * 512, (c + 1) * 512)
            aot = [apool.tile([128, 512], BF16, tag=f"aot{k}", name=f"aot{c}_{k}")
                   for k in range(4)]
            npairs = 2 * (c + 1)
            for h in range(8):
                ot = psot.tile([66, 512], F32, tag="ot", name=f"ot{c}_{h}")
                if c == 0:
                    _attn_head_ch0(h, ot)
                else:
                    for tp in range(npairs):
                        diag = tp >= npairs - 2
                        U = 256 if (diag and tp == npairs - 1) else 512
                        u0 = 512 - U
                        quad = psq.tile([128, 1024], F32, tag="quad",
                                        name=f"q{c}_{h}_{tp}")
                        et = epool.tile([128, 1024], F8, tag="e8",
                                        name=f"e{c}_{h}_{tp}")
                        scores_f8(h, quad, et, tp, c * 512 + u0, U, diag)
                        exp_quad(et, quad, U, diag, split=True, mask=tk8)
                        nc.tensor.matmul(
                            ot[:, u0:u0 + U],
                            lhsT=vt8[tp][:, h, :, 0:66],
                            rhs=xr(et[:, 0:2 * U], "p (s u) -> p s u", s=2),
                            start=(tp == 0), stop=(tp == npairs - 1),
                            perf_mode=DR)
                # normalize: row 64 of ot is the denominator
                den = spool.tile([1, 512], F32, tag="den", name=f"dn{c}_{h}")
                nc.vector.tensor_copy(den, ot[64:65, :])
                recf = spool.tile([1, 512], F32, tag="recf", name=f"rf{c}_{h}")
                nc.vector.reciprocal_approx_fast(recf, den)
                recb = spool.tile([1, 512], BF16, tag="recb", name=f"rb{c}_{h}")
                nc.vector.tensor_copy(recb, recf)
                dr = dpool.tile([1, 512], BF16, tag="dr", name=f"dr{c}_{h}")
                nc.sync.dma_start(out=dr, in_=recb)
                bcs = spool.tile([64, 512], BF16, tag="bcs", bufs=2,
                                 name=f"bs{c}_{h}")
                nc.sync.dma_start(out=bcs, in_=dr.to_broadcast((64, 512)))
                nc.vector.tensor_mul(
                    aot[h // 2][(h % 2) * 64:(h % 2) * 64 + 64, :],
                    ot[0:64, :], bcs)
                # pump fillers so the PE queue has QKV/out-proj work while
                # the exp + normalize chains of this head drain
                for _ in range(3):
                    if fillers:
                        fillers.popleft()()
            while fillers:
                fillers.popleft()()
            return aot

        def _attn_head_ch0(h, ot):
            # block A: queries [0, BFQ) in bf16 over key-tiles 0..BFQ/128-1
            UA = BFQ // 2  # window width per sub (2 ktiles span BFQ keys)
            quad = psq.tile([128, 1024], F32, tag="quad", name=f"qA_{h}")
            etA = epool.tile([128, 1024], BF16, tag="eb", name=f"eA_{h}")
            scores_bf(h, quad, 0, BFQ, diag=True)
            exp_quad(etA, quad, BFQ, diag=True, split=True, mask=tkb)
            nc.tensor.matmul(
                ot[0:65, 0:BFQ], lhsT=vtb[0][:, h, :], rhs=etA[:, 0:BFQ],
                start=True, stop=False, skip_group_check=True)
            nc.tensor.matmul(
                ot[0:65, 128:BFQ], lhsT=vtb[1][:, h, :],
                rhs=etA[:, BFQ + 128:2 * BFQ],
                start=False, stop=False, skip_group_check=True)
            # block B: queries [BFQ, 512) in fp8 over all 4 chunk-0 ktiles
            UB = 512 - BFQ
            for tp in range(2):
                diag = tp == 1  # ktiles 2,3 overlap the B query range
                quad = psq.tile([128, 1024], F32, tag="quad", name=f"qB_{h}_{tp}")
                et = epool.tile([128, 1024], F8, tag="e8", name=f"eB_{h}_{tp}")
                scores_f8(h, quad, et, tp, BFQ, UB, diag)
                exp_quad(et, quad, UB, diag, split=True, mask=tk8)
                nc.tensor.matmul(
                    ot[:, BFQ:512],
                    lhsT=vt8[tp][:, h, :, 0:66],
                    rhs=xr(et[:, 0:2 * UB], "p (s u) -> p s u", s=2),
                    start=False, stop=(tp == 1), skip_group_check=True,
                    perf_mode=DR)

        def stage3_fillers(c, aot):
            cs = slice(c * 512, (c + 1) * 512)
            fillers = []

            def out_group(od):
                def go():
                    # for the final chunk the score-quad banks are free:
                    # alternate pools so consecutive groups double-buffer
                    if c == NCH - 1 and od % 2 == 1:
                        ps = psq.tile([128, 1024], F32, tag="quad",
                                      name=f"py{c}_{od}")[:, 0:512]
                    else:
                        ps = psmm.tile([128, 512], F32, tag="qkv",
                                       name=f"py{c}_{od}")
                    for k in range(4):
                        nc.tensor.matmul(
                            ps,
                            lhsT=wo[k][:, od * 128:(od + 1) * 128],
                            rhs=aot[k],
                            start=(k == 0), stop=(k == 3))
                    ys = spool.tile([128, 512], BF16, tag="ys", bufs=2,
                                    name=f"ys{c}_{od}")
                    if c == NCH - 1 and od >= 5:
                        nc.scalar.copy(ys, ps)   # ACT is idle at the tail
                    else:
                        nc.vector.tensor_copy(ys, ps)
                    nc.sync.dma_start(out=yt[od * 128:(od + 1) * 128, cs], in_=ys)
                return go

            for od in range(8):
                fillers.append(out_group(od))
            return fillers

        # ---- schedule ----
        # stage-3 fillers are deferred a chunk so the later (bigger)
        # attention chunks keep the PE fed while exp chains drain
        stage1_ch0()
        s3 = {}
        for c in range(NCH):
            fl = deque()
            if c + 1 < NCH:
                fl.extend(stage1_fp8_fillers(c + 1))
            if c == 1:
                fl.extend(s3[0])
            elif c == 2:
                fl.extend(s3[1][:4])
            elif c == 3:
                fl.extend(s3[1][4:])
                fl.extend(s3[2])
            aot = attention(c, fl)
            s3[c] = stage3_fillers(c, aot)
        for f in s3[NCH - 1]:
            f()


_NC_CACHE = None


def _get_nc():
    global _NC_CACHE
    if _NC_CACHE is None:
        _NC_CACHE = build_bass()
    return _NC_CACHE


def _keep_pattern():
    k = np.arange(128)[:, None]
    q = np.arange(128)[None, :]
    return (q >= k).astype(np.float32)        # 1 = attended


def make_in_maps(x, w_qkv, w_out):
    x = np.asarray(x, dtype=np.float32)
    w_qkv = np.asarray(w_qkv, dtype=np.float32)
    w_out = np.asarray(w_out, dtype=np.float32)
    pat = _keep_pattern()
    f8 = ml_dtypes.float8_e4m3
    bf = ml_dtypes.bfloat16

    in_maps = []
    for core in range(N_CORES):
        b, g = core // 2, core % 2
        gs = slice(g * HG, (g + 1) * HG)
        wsel = np.concatenate(
            [w_qkv[0 * INNER:][gs], w_qkv[1 * INNER:][gs], w_qkv[2 * INNER:][gs]],
            axis=0)                               # [1536, 1024] bf16 order
        # fp8 weight column order: q/k in (grp, dim-half) blocks of 4h x 32d
        cols = np.empty(3 * HG, np.int64)
        j = np.arange(HG)
        m, r = j // 128, j % 128
        hh = g * 8 + (m // 2) * 4 + r // 32
        d = (m % 2) * 32 + (r % 32)
        cols[0:HG] = hh * 64 + d
        cols[HG:2 * HG] = INNER + hh * 64 + d
        cols[2 * HG:] = 2 * INNER + (g * 8 + j // 64) * 64 + (j % 64)
        wsel8 = w_qkv[cols, :]                    # [1536, 1024]
        # wq8 dram [512, 3072]: row 128k+p, col i*1536+j = wsel8[j, 256k+128i+p]
        wq8d = wsel8.T.reshape(4, 2, 128, 3 * HG).transpose(0, 2, 1, 3)
        wq8d = np.ascontiguousarray(wq8d.reshape(512, 2 * 3 * HG))
        # xt8 dram [512, 4096]: row 128k+p, col i*2048+t = x[b][t, 256k+128i+p]
        xt8d = x[b].T.reshape(4, 2, 128, T).transpose(0, 2, 1, 3)
        xt8d = np.ascontiguousarray(xt8d.reshape(512, 2 * T))
        in_maps.append({
            "xtbf": np.ascontiguousarray(x[b][0:BFQ].T).astype(bf),
            "xt8": xt8d.astype(f8),
            "wqbf": np.ascontiguousarray(wsel.T).astype(bf),
            "wq8": wq8d.astype(f8),
            "woutt": np.ascontiguousarray(w_out[:, gs].T).astype(bf),
            "trik8": pat.astype(f8),
            "trikb": pat.astype(bf),
        })
    return in_maps


def kernel(x, mask, w_qkv, w_out, **_):
    nc = _get_nc()
    in_maps = make_in_maps(x, w_qkv, w_out)
    res = run_bass_kernel_spmd(nc, in_maps, core_ids=list(range(N_CORES)))
    y = np.zeros((B, T, DIM), dtype=np.float32)
    for c in range(N_CORES):
        y[c // 2] += res.results[c]["yt"].astype(np.float32).T
    return y


# revision 23
# speedup vs baseline: 1.0417x; 1.0091x over previous
"""Causal self-attention Trainium2 Bass kernel (fp8 DoubleRow version).

Problem: B=4, T=2048, DIM=1024, H=16 heads, head_dim=64 (fp32).
  qkv = x @ w_qkv.T ; per-head causal softmax(q k^T / 8) v ; out @ w_out.T

Sharding (8 cores): core c -> (batch b = c//2, head-group g = c%2 of 8 heads).
Each core computes a partial output y_partial = attn_out_g @ w_out[:, g]^T
for its batch; host sums the two head-group partials per batch.

Precision scheme (validated vs fp32 reference, sim rel max err 0.0115 < 2e-2):
  - QKV projection in fp8e4 (e4m3) with MatmulPerfMode.DoubleRow: each
    matmul contracts 2 k-subtiles of 128 per pass.
  - Scores in fp8 DoubleRow: q/k stored [32 part, 2 dim-half, tokens]
    per 4-head group (weight columns pre-ordered on host), K = 32 x 2.
  - PV in fp8 DoubleRow over key-tile pairs: lhsT = v [128, 2, 66]
    (col 64 of ones emits the softmax denominator row; col 65 zero pad;
    sub-stride 80 because dual-fp8 LDWEIGHTS requires step % 16 == 0).
  - exp computes exp(s/8 - 2): the -2 bias keeps e < 240 (fp8 max);
    it cancels in the normalization.
  - Queries 0:256 (few attended keys -> fp8 noise doesn't average out)
    use a bf16 path: bf16 QKV over tokens 0:256 + bf16 scores/PV.
  - Output projection in bf16.

Causal masking: work above the diagonal is skipped.  Diagonal quads
slice scores/exp to the live query range per key-subtile; the
fully-dead leading block of each odd subtile is never computed (its e
range is memset to 0 for the fp8 PV; the bf16 PV just slices it out);
the 128x128 boundary triangle of each subtile is zeroed by a keep-mask
multiply on the (otherwise idle) GpSimd/Pool engine after exp.

The PE work is interleaved: QKV projection ("stage 1") of chunk c+1
and output projection ("stage 3") of earlier chunks are pumped as
fillers between attention heads, so the tensor engine stays busy while
exp/normalize chains drain (idle PE gaps also drop the HAM clock gate
to half rate, so continuous occupancy matters twice).
"""

import contextlib
from collections import deque

import numpy as np
import ml_dtypes

import concourse.mybir as mybir
import concourse.tile as tile
from concourse import bacc
from concourse.bass_utils import run_bass_kernel_spmd

B, T, DIM = 4, 2048, 1024
NUM_HEADS, HEAD_DIM = 16, 64
INNER = NUM_HEADS * HEAD_DIM
SCALE = HEAD_DIM ** -0.5

N_CORES = 8
HEADS_PER_CORE = 8
HG = HEADS_PER_CORE * HEAD_DIM  # 512 = inner slice per core
NCH = T // 512                  # 4 token chunks

F32 = mybir.dt.float32
BF16 = mybir.dt.bfloat16
F8 = mybir.dt.float8e4
DR = mybir.MatmulPerfMode.DoubleRow
EXPF = mybir.ActivationFunctionType.Exp

EXP_BIAS = -2.0
BFQ = 256          # queries [0, BFQ) take the bf16 path


def xr(ap, pattern, **kw):
    return ap.rearrange(pattern, **kw)


def build_bass():
    nc = bacc.Bacc()
    xtbf = nc.declare_dram_parameter("xtbf", [DIM, BFQ], BF16, isOutput=False)
    xt8 = nc.declare_dram_parameter("xt8", [512, 2 * T], F8, isOutput=False)
    wqbf = nc.declare_dram_parameter("wqbf", [DIM, 3 * HG], BF16, isOutput=False)
    wq8 = nc.declare_dram_parameter("wq8", [512, 2 * 3 * HG], F8, isOutput=False)
    woutt = nc.declare_dram_parameter("woutt", [HG, DIM], BF16, isOutput=False)
    trik8 = nc.declare_dram_parameter("trik8", [128, 128], F8, isOutput=False)
    trikb = nc.declare_dram_parameter("trikb", [128, 128], BF16, isOutput=False)
    yt = nc.declare_dram_parameter("yt", [DIM, T], BF16, isOutput=True)

    with tile.TileContext(nc) as tc:
        _emit(nc, tc, xtbf, xt8, wqbf, wq8, woutt, trik8, trikb, yt)
    nc.finalize()
    return nc


def _emit(nc, tc, xtbf, xt8, wqbf, wq8, woutt, trik8, trikb, yt):
    ctx = contextlib.ExitStack()
    with ctx:
        singles = ctx.enter_context(tc.tile_pool(name="singles", bufs=1))
        xpool = ctx.enter_context(tc.tile_pool(name="xpool", bufs=2))
        epool = ctx.enter_context(tc.tile_pool(name="epool", bufs=3))
        apool = ctx.enter_context(tc.tile_pool(name="apool", bufs=3))
        spool = ctx.enter_context(tc.tile_pool(name="spool", bufs=1))
        dpool = ctx.enter_context(tc.tile_pool(name="dpool", bufs=2, space="DRAM"))
        # PSUM budget (8 banks of 2KB/partition):
        #   psq  [128,1024] bufs=2 -> 4 banks (score quads, double-buffered)
        #   psot [66,512]   bufs=3 -> 3 banks (per-head PV accumulators)
        #   psmm [128,512]  bufs=1 -> 1 bank (stage 1 + stage 3 groups)
        psq = ctx.enter_context(tc.tile_pool(name="psq", bufs=2, space="PSUM"))
        psot = ctx.enter_context(tc.tile_pool(name="psot", bufs=3, space="PSUM"))
        psmm = ctx.enter_context(tc.tile_pool(name="psmm", bufs=1, space="PSUM"))

        # ---- persistent SBUF tensors; DMA order = dependency order ----
        # interleave weight/x DMAs so the first stage-1 group can start after
        # only one (w, x) tile pair has landed
        wqb = []
        xts0 = []
        for k in range(8):
            w = singles.tile([128, 3 * HG], BF16, name=f"wqb{k}")
            nc.sync.dma_start(out=w[:, 0:HG], in_=wqbf[k * 128:(k + 1) * 128, 0:HG])
            wqb.append(w)
            xtile = xpool.tile([128, BFQ], BF16, tag=f"xb{k}", name=f"xb{k}")
            nc.sync.dma_start(out=xtile, in_=xtbf[k * 128:(k + 1) * 128, :])
            xts0.append(xtile)
        for k in range(8):
            nc.sync.dma_start(out=wqb[k][:, HG:3 * HG],
                              in_=wqbf[k * 128:(k + 1) * 128, HG:3 * HG])
        wq8s = []
        for k in range(4):
            w = singles.tile([128, 2, 3 * HG], F8, name=f"wq8{k}")
            nc.sync.dma_start(out=xr(w, "p s c -> p (s c)"),
                              in_=wq8[k * 128:(k + 1) * 128, :])
            wq8s.append(w)

        def x8_tiles(c):
            cs = slice(c * 512, (c + 1) * 512)
            x8s = []
            for k in range(4):
                t8 = xpool.tile([128, 2, 512], F8, tag=f"x8_{k}",
                                name=f"x8_{c}_{k}")
                nc.sync.dma_start(
                    out=t8,
                    in_=xr(xt8[k * 128:(k + 1) * 128, :],
                           "p (s t) -> p s t", s=2)[:, :, cs])
                x8s.append(t8)
            return x8s

        x8s0 = x8_tiles(0)

        wo = []
        for k in range(4):
            w = singles.tile([128, DIM], BF16, name=f"wo{k}")
            nc.sync.dma_start(out=w, in_=woutt[k * 128:(k + 1) * 128, :])
            wo.append(w)
        tk8 = singles.tile([128, 128], F8, name="tk8")
        nc.sync.dma_start(out=tk8, in_=trik8[:, :])
        tkb = singles.tile([128, 128], BF16, name="tkb")
        nc.sync.dma_start(out=tkb, in_=trikb[:, :])
        biasap = singles.tile([128, 1], F32, name="expbias")
        nc.gpsimd.memset(biasap, EXP_BIAS)

        # bf16 q/k for queries/keys [0, BFQ): 4 tiles [128, BFQ]
        qtb = [singles.tile([128, BFQ], BF16, name=f"qtb{m}") for m in range(4)]
        ktb = [singles.tile([128, BFQ], BF16, name=f"ktb{m}") for m in range(4)]
        # bf16 v for keys [0, BFQ): 2 token-tiles [128, 8, 65]
        vtb = [singles.tile([128, HEADS_PER_CORE, 65], BF16, name=f"vtb{t}")
               for t in range(BFQ // 128)]
        for t in range(BFQ // 128):
            nc.gpsimd.memset(vtb[t][:, :, 64:65], 1.0)
        # fp8 q for tokens [BFQ, T): [4h x 32d, 2 dim-half, T - BFQ]
        qt8 = [singles.tile([128, 2, T - BFQ], F8, name=f"qt8{g}")
               for g in range(2)]
        # fp8 k (all tokens): [128, 2, 2048]
        kt8 = [singles.tile([128, 2, T], F8, name=f"kt8{g}") for g in range(2)]
        # fp8 v: 8 ktile-pair tiles; inner stride 80 (dual-fp8 LDW needs
        # the sub-ktile step to be a multiple of 16), 66 cols used
        vt8 = [singles.tile([128, HEADS_PER_CORE, 2, 80], F8, name=f"vt8{tp}")
               for tp in range(8)]
        for tp in range(8):
            nc.gpsimd.memset(vt8[tp][:, :, :, 64:65], 1.0)
            nc.gpsimd.memset(vt8[tp][:, :, :, 65:66], 0.0)

        def stage1_ch0():
            """Chunk 0: bf16 QKV over [0,BFQ) + fp8 q/k/v as needed."""
            # bf16 q, k over tokens [0, BFQ)
            for which, dst in ((0, qtb), (1, ktb)):
                for m in range(4):
                    ps = psmm.tile([128, 512], F32, tag="qkv",
                                   name=f"pb{which}{m}")[:, 0:BFQ]
                    for k in range(8):
                        nc.tensor.matmul(
                            ps,
                            lhsT=wqb[k][:, which * HG + m * 128:
                                        which * HG + (m + 1) * 128],
                            rhs=xts0[k],
                            start=(k == 0), stop=(k == 7))
                    nc.vector.tensor_copy(dst[m], ps)
            # bf16 v over token tiles [0, BFQ) + fp8 casts of the same psum
            for i in range(BFQ // 128):
                ps = psmm.tile([128, 512], F32, tag="qkv", name=f"pbv{i}")
                for k in range(8):
                    nc.tensor.matmul(
                        ps,
                        lhsT=xts0[k][:, i * 128:(i + 1) * 128],
                        rhs=wqb[k][:, 2 * HG:3 * HG],
                        start=(k == 0), stop=(k == 7))
                ps3 = xr(ps, "p (h d) -> p h d", h=HEADS_PER_CORE)
                nc.vector.tensor_copy(vtb[i][:, :, 0:64], ps3)
                nc.vector.tensor_copy(vt8[i // 2][:, :, i % 2, 0:64], ps3)
            # fp8 k for chunk 0 (all 512 keys)
            for m in range(4):
                ps = psmm.tile([128, 512], F32, tag="qkv", name=f"p8k{m}")
                for k in range(4):
                    nc.tensor.matmul(
                        ps,
                        lhsT=wq8s[k][:, :, HG + m * 128:HG + (m + 1) * 128],
                        rhs=x8s0[k],
                        start=(k == 0), stop=(k == 3), perf_mode=DR)
                nc.vector.tensor_copy(kt8[m // 2][:, m % 2, 0:512], ps)
            # fp8 q for tokens [BFQ, 512)
            for m in range(4):
                ps = psmm.tile([128, 512], F32, tag="qkv",
                               name=f"p8q{m}")[:, 0:512 - BFQ]
                for k in range(4):
                    nc.tensor.matmul(
                        ps,
                        lhsT=wq8s[k][:, :, m * 128:(m + 1) * 128],
                        rhs=x8s0[k][:, :, BFQ:512],
                        start=(k == 0), stop=(k == 3), perf_mode=DR)
                nc.vector.tensor_copy(qt8[m // 2][:, m % 2, 0:512 - BFQ], ps)
            # fp8 v for token tiles [BFQ, 512)
            for i in range(BFQ // 128, 4):
                ps = psmm.tile([128, 512], F32, tag="qkv", name=f"p8v{i}")
                for k in range(4):
                    nc.tensor.matmul(
                        ps,
                        lhsT=x8s0[k][:, :, i * 128:(i + 1) * 128],
                        rhs=wq8s[k][:, :, 2 * HG:3 * HG],
                        start=(k == 0), stop=(k == 3), perf_mode=DR)
                nc.vector.tensor_copy(
                    vt8[i // 2][:, :, i % 2, 0:64],
                    xr(ps, "p (h d) -> p h d", h=HEADS_PER_CORE))

        def stage1_fp8_fillers(c):
            """fp8 QKV for chunk c>=1, as filler closures (4 matmuls each)."""
            cs = slice(c * 512, (c + 1) * 512)
            qcs = slice(c * 512 - BFQ, (c + 1) * 512 - BFQ)
            x8s = x8_tiles(c)
            fillers = []

            def qk_group(which, m):
                def go():
                    ps = psmm.tile([128, 512], F32, tag="qkv",
                                   name=f"p8{which}_{c}_{m}")
                    for k in range(4):
                        nc.tensor.matmul(
                            ps,
                            lhsT=wq8s[k][:, :, which * HG + m * 128:
                                         which * HG + (m + 1) * 128],
                            rhs=x8s[k],
                            start=(k == 0), stop=(k == 3), perf_mode=DR)
                    dst = (qt8 if which == 0 else kt8)[m // 2]
                    nc.vector.tensor_copy(
                        dst[:, m % 2, qcs if which == 0 else cs], ps)
                return go

            def v_group(i):
                def go():
                    t = 4 * c + i
                    ps = psmm.tile([128, 512], F32, tag="qkv", name=f"p8V{c}{i}")
                    for k in range(4):
                        nc.tensor.matmul(
                            ps,
                            lhsT=x8s[k][:, :, i * 128:(i + 1) * 128],
                            rhs=wq8s[k][:, :, 2 * HG:3 * HG],
                            start=(k == 0), stop=(k == 3), perf_mode=DR)
                    nc.vector.tensor_copy(
                        vt8[t // 2][:, :, t % 2, 0:64],
                        xr(ps, "p (h d) -> p h d", h=HEADS_PER_CORE))
                return go

            for m in range(4):
                fillers.append(qk_group(1, m))
            for i in range(4):
                fillers.append(v_group(i))
            for m in range(4):
                fillers.append(qk_group(0, m))
            return fillers

        # ---------------- attention building blocks ----------------
        # A quad holds scores for (head h, key-tiles 2tp and 2tp+1) over a
        # query window of width U: quad cols [sub*U, sub*U + U).
        # "live0" = first live query col (relative to the window) of sub0;
        # sub1's live range starts 128 later.  Window col 0 = query qlo.

        def scores_f8(h, quad, et, tp, qlo, U, diag):
            g, hi = h // 4, 32 * (h % 4)
            qg = qt8[g]
            qoff = qlo - BFQ
            for sub in range(2):
                t = 2 * tp + sub
                lhsT = kt8[g][hi:hi + 32, :, t * 128:(t + 1) * 128]
                base = sub * U
                if not diag:
                    nc.tensor.matmul(
                        quad[:, base:base + U],
                        lhsT=lhsT, rhs=qg[hi:hi + 32, :, qoff:qoff + U],
                        start=True, stop=True, tile_position=(hi, 0),
                        perf_mode=DR)
                    continue
                lo = sub * 128          # dead cols [0, lo) for sub1
                if sub == 1:
                    nc.gpsimd.memset(et[:, U:U + 128], 0.0)
                nc.tensor.matmul(
                    quad[:, base + lo:base + U],
                    lhsT=lhsT, rhs=qg[hi:hi + 32, :, qoff + lo:qoff + U],
                    start=True, stop=True, tile_position=(hi, 0), perf_mode=DR)

        def scores_bf(h, quad, tp, U, diag):
            m, ho = h // 2, 64 * (h % 2)
            for sub in range(2):
                t = 2 * tp + sub
                lhsT = ktb[m][ho:ho + 64, t * 128:(t + 1) * 128]
                base = sub * U
                lo = sub * 128 if diag else 0
                nc.tensor.matmul(
                    quad[:, base + lo:base + U],
                    lhsT=lhsT, rhs=qtb[m][ho:ho + 64, lo:U],
                    start=True, stop=True, tile_position=(ho, 0))

        def exp_quad(et, quad, U, diag, split, mask):
            if diag and split:
                nc.scalar.activation(et[:, 0:U], quad[:, 0:U], EXPF,
                                     scale=float(SCALE), bias=biasap)
                nc.scalar.activation(et[:, U + 128:2 * U], quad[:, U + 128:2 * U],
                                     EXPF, scale=float(SCALE), bias=biasap)
                # boundary triangles: keep-mask on the (otherwise idle) Pool
                nc.gpsimd.tensor_mul(et[:, 0:128], et[:, 0:128], mask)
                nc.gpsimd.tensor_mul(et[:, U + 128:U + 256],
                                     et[:, U + 128:U + 256], mask)
            else:
                nc.scalar.activation(et[:, 0:2 * U], quad[:, 0:2 * U], EXPF,
                                     scale=float(SCALE), bias=biasap)

        def attention(c, fillers):
            cs = slice(c * 512, (c + 1) * 512)
            aot = [apool.tile([128, 512], BF16, tag=f"aot{k}", name=f"aot{c}_{k}")
                   for k in range(4)]
            npairs = 2 * (c + 1)
            for h in range(8):
                ot = psot.tile([66, 512], F32, tag="ot", name=f"ot{c}_{h}")
                if c == 0:
                    _attn_head_ch0(h, ot)
                else:
                    for tp in range(npairs):
                        diag = tp >= npairs - 2
                        U = 256 if (diag and tp == npairs - 1) else 512
                        u0 = 512 - U
                        quad = psq.tile([128, 1024], F32, tag="quad",
                                        name=f"q{c}_{h}_{tp}")
                        et = epool.tile([128, 1024], F8, tag="e8",
                                        name=f"e{c}_{h}_{tp}")
                        scores_f8(h, quad, et, tp, c * 512 + u0, U, diag)
                        exp_quad(et, quad, U, diag, split=True, mask=tk8)
                        nc.tensor.matmul(
                            ot[:, u0:u0 + U],
                            lhsT=vt8[tp][:, h, :, 0:66],
                            rhs=xr(et[:, 0:2 * U], "p (s u) -> p s u", s=2),
                            start=(tp == 0), stop=(tp == npairs - 1),
                            perf_mode=DR)
                # normalize: row 64 of ot is the denominator
                den = spool.tile([1, 512], F32, tag="den", name=f"dn{c}_{h}")
                nc.vector.tensor_copy(den, ot[64:65, :])
                recf = spool.tile([1, 512], F32, tag="recf", name=f"rf{c}_{h}")
                nc.vector.reciprocal_approx_fast(recf, den)
                recb = spool.tile([1, 512], BF16, tag="recb", name=f"rb{c}_{h}")
                nc.vector.tensor_copy(recb, recf)
                dr = dpool.tile([1, 512], BF16, tag="dr", name=f"dr{c}_{h}")
                nc.sync.dma_start(out=dr, in_=recb)
                bcs = spool.tile([64, 512], BF16, tag="bcs", bufs=2,
                                 name=f"bs{c}_{h}")
                nc.sync.dma_start(out=bcs, in_=dr.to_broadcast((64, 512)))
                nc.vector.tensor_mul(
                    aot[h // 2][(h % 2) * 64:(h % 2) * 64 + 64, :],
                    ot[0:64, :], bcs)
                # pump fillers so the PE queue has QKV/out-proj work while
                # the exp + normalize chains of this head drain
                for _ in range(2):
                    if fillers:
                        fillers.popleft()()
            while fillers:
                fillers.popleft()()
            return aot

        def _attn_head_ch0(h, ot):
            # block A: queries [0, BFQ) in bf16 over key-tiles 0..BFQ/128-1
            UA = BFQ // 2  # window width per sub (2 ktiles span BFQ keys)
            quad = psq.tile([128, 1024], F32, tag="quad", name=f"qA_{h}")
            etA = epool.tile([128, 1024], BF16, tag="eb", name=f"eA_{h}")
            scores_bf(h, quad, 0, BFQ, diag=True)
            exp_quad(etA, quad, BFQ, diag=True, split=True, mask=tkb)
            nc.tensor.matmul(
                ot[0:65, 0:BFQ], lhsT=vtb[0][:, h, :], rhs=etA[:, 0:BFQ],
                start=True, stop=False, skip_group_check=True)
            nc.tensor.matmul(
                ot[0:65, 128:BFQ], lhsT=vtb[1][:, h, :],
                rhs=etA[:, BFQ + 128:2 * BFQ],
                start=False, stop=False, skip_group_check=True)
            # block B: queries [BFQ, 512) in fp8 over all 4 chunk-0 ktiles
            UB = 512 - BFQ
            for tp in range(2):
                diag = tp == 1  # ktiles 2,3 overlap the B query range
                quad = psq.tile([128, 1024], F32, tag="quad", name=f"qB_{h}_{tp}")
                et = epool.tile([128, 1024], F8, tag="e8", name=f"eB_{h}_{tp}")
                scores_f8(h, quad, et, tp, BFQ, UB, diag)
                exp_quad(et, quad, UB, diag, split=True, mask=tk8)
                nc.tensor.matmul(
                    ot[:, BFQ:512],
                    lhsT=vt8[tp][:, h, :, 0:66],
                    rhs=xr(et[:, 0:2 * UB], "p (s u) -> p s u", s=2),
                    start=False, stop=(tp == 1), skip_group_check=True,
                    perf_mode=DR)

        def stage3_fillers(c, aot):
            cs = slice(c * 512, (c + 1) * 512)
            fillers = []

            def out_group(od):
                def go():
                    # for the final chunk the score-quad banks are free:
                    # alternate pools so consecutive groups double-buffer
                    if c == NCH - 1 and od % 2 == 1:
                        ps = psq.tile([128, 1024], F32, tag="quad",
                                      name=f"py{c}_{od}")[:, 0:512]
                    else:
                        ps = psmm.tile([128, 512], F32, tag="qkv",
                                       name=f"py{c}_{od}")
                    for k in range(4):
                        nc.tensor.matmul(
                            ps,
                            lhsT=wo[k][:, od * 128:(od + 1) * 128],
                            rhs=aot[k],
                            start=(k == 0), stop=(k == 3))
                    ys = spool.tile([128, 512], BF16, tag="ys", bufs=2,
                                    name=f"ys{c}_{od}")
                    if c == NCH - 1 and od >= 5:
                        nc.scalar.copy(ys, ps)   # ACT is idle at the tail
                    else:
                        nc.vector.tensor_copy(ys, ps)
                    nc.sync.dma_start(out=yt[od * 128:(od + 1) * 128, cs], in_=ys)
                return go

            for od in range(8):
                fillers.append(out_group(od))
            return fillers

        # ---- schedule ----
        # stage-3 fillers are deferred a chunk so the later (bigger)
        # attention chunks keep the PE fed while exp chains drain
        stage1_ch0()
        s3 = {}
        for c in range(NCH):
            fl = deque()
            if c + 1 < NCH:
                fl.extend(stage1_fp8_fillers(c + 1))
            if c == 1:
                fl.extend(s3[0])
            elif c == 2:
                fl.extend(s3[1][:4])
            elif c == 3:
                fl.extend(s3[1][4:])
                fl.extend(s3[2])
            aot = attention(c, fl)
            s3[c] = stage3_fillers(c, aot)
        for f in s3[NCH - 1]:
            f()


_NC_CACHE = None


def _get_nc():
    global _NC_CACHE
    if _NC_CACHE is None:
        _NC_CACHE = build_bass()
    return _NC_CACHE


def _keep_pattern():
    k = np.arange(128)[:, None]
    q = np.arange(128)[None, :]
    return (q >= k).astype(np.float32)        # 1 = attended


def make_in_maps(x, w_qkv, w_out):
    x = np.asarray(x, dtype=np.float32)
    w_qkv = np.asarray(w_qkv, dtype=np.float32)
    w_out = np.asarray(w_out, dtype=np.float32)
    pat = _keep_pattern()
    f8 = ml_dtypes.float8_e4m3
    bf = ml_dtypes.bfloat16

    in_maps = []
    for core in range(N_CORES):
        b, g = core // 2, core % 2
        gs = slice(g * HG, (g + 1) * HG)
        wsel = np.concatenate(
            [w_qkv[0 * INNER:][gs], w_qkv[1 * INNER:][gs], w_qkv[2 * INNER:][gs]],
            axis=0)                               # [1536, 1024] bf16 order
        # fp8 weight column order: q/k in (grp, dim-half) blocks of 4h x 32d
        cols = np.empty(3 * HG, np.int64)
        j = np.arange(HG)
        m, r = j // 128, j % 128
        hh = g * 8 + (m // 2) * 4 + r // 32
        d = (m % 2) * 32 + (r % 32)
        cols[0:HG] = hh * 64 + d
        cols[HG:2 * HG] = INNER + hh * 64 + d
        cols[2 * HG:] = 2 * INNER + (g * 8 + j // 64) * 64 + (j % 64)
        wsel8 = w_qkv[cols, :]                    # [1536, 1024]
        # wq8 dram [512, 3072]: row 128k+p, col i*1536+j = wsel8[j, 256k+128i+p]
        wq8d = wsel8.T.reshape(4, 2, 128, 3 * HG).transpose(0, 2, 1, 3)
        wq8d = np.ascontiguousarray(wq8d.reshape(512, 2 * 3 * HG))
        # xt8 dram [512, 4096]: row 128k+p, col i*2048+t = x[b][t, 256k+128i+p]
        xt8d = x[b].T.reshape(4, 2, 128, T).transpose(0, 2, 1, 3)
        xt8d = np.ascontiguousarray(xt8d.reshape(512, 2 * T))
        in_maps.append({
            "xtbf": np.ascontiguousarray(x[b][0:BFQ].T).astype(bf),
            "xt8": xt8d.astype(f8),
            "wqbf": np.ascontiguousarray(wsel.T).astype(bf),
            "wq8": wq8d.astype(f8),
            "woutt": np.ascontiguousarray(w_out[:, gs].T).astype(bf),
            "trik8": pat.astype(f8),
            "trikb": pat.astype(bf),
        })
    return in_maps


def kernel(x, mask, w_qkv, w_out, **_):
    nc = _get_nc()
    in_maps = make_in_maps(x, w_qkv, w_out)
    res = run_bass_kernel_spmd(nc, in_maps, core_ids=list(range(N_CORES)))
    y = np.zeros((B, T, DIM), dtype=np.float32)
    for c in range(N_CORES):
        y[c // 2] += res.results[c]["yt"].astype(np.float32).T
    return y


# revision 24
# speedup vs baseline: 1.0829x; 1.0396x over previous
"""Causal self-attention Trainium2 Bass kernel (fp8 DoubleRow version).

Problem: B=4, T=2048, DIM=1024, H=16 heads, head_dim=64 (fp32).
  qkv = x @ w_qkv.T ; per-head causal softmax(q k^T / 8) v ; out @ w_out.T

Sharding (8 cores): core c -> (batch b = c//2, head-group g = c%2 of 8 heads).
Each core computes a partial output y_partial = attn_out_g @ w_out[:, g]^T
for its batch; host sums the two head-group partials per batch.

Precision scheme (validated vs fp32 reference, sim rel max err 0.0115 < 2e-2):
  - QKV projection in fp8e4 (e4m3) with MatmulPerfMode.DoubleRow: each
    matmul contracts 2 k-subtiles of 128 per pass.
  - Scores in fp8 DoubleRow: q/k stored [32 part, 2 dim-half, tokens]
    per 4-head group (weight columns pre-ordered on host), K = 32 x 2.
  - PV in fp8 DoubleRow over key-tile pairs: lhsT = v [128, 2, 66]
    (col 64 of ones emits the softmax denominator row; col 65 zero pad;
    sub-stride 80 because dual-fp8 LDWEIGHTS requires step % 16 == 0).
  - exp computes exp(s/8 - 2): the -2 bias keeps e < 240 (fp8 max);
    it cancels in the normalization.
  - Queries 0:256 (few attended keys -> fp8 noise doesn't average out)
    use a bf16 path: bf16 QKV over tokens 0:256 + bf16 scores/PV.
  - Output projection in bf16.

Causal masking: work above the diagonal is skipped.  Diagonal quads
slice scores/exp to the live query range per key-subtile; the
fully-dead leading block of each odd subtile is never computed (its e
range is memset to 0 for the fp8 PV; the bf16 PV just slices it out);
the 128x128 boundary triangle of each subtile is zeroed by a keep-mask
multiply on the (otherwise idle) GpSimd/Pool engine after exp.

The PE work is interleaved: QKV projection ("stage 1") of chunk c+1
and output projection ("stage 3") of earlier chunks are pumped as
fillers between attention heads, so the tensor engine stays busy while
exp/normalize chains drain (idle PE gaps also drop the HAM clock gate
to half rate, so continuous occupancy matters twice).
"""

import contextlib
from collections import deque

import numpy as np
import ml_dtypes

import concourse.mybir as mybir
import concourse.tile as tile
from concourse import bacc
from concourse.bass_utils import run_bass_kernel_spmd

B, T, DIM = 4, 2048, 1024
NUM_HEADS, HEAD_DIM = 16, 64
INNER = NUM_HEADS * HEAD_DIM
SCALE = HEAD_DIM ** -0.5

N_CORES = 8
HEADS_PER_CORE = 8
HG = HEADS_PER_CORE * HEAD_DIM  # 512 = inner slice per core
NCH = T // 512                  # 4 token chunks

F32 = mybir.dt.float32
BF16 = mybir.dt.bfloat16
F8 = mybir.dt.float8e4
DR = mybir.MatmulPerfMode.DoubleRow
EXPF = mybir.ActivationFunctionType.Exp

EXP_BIAS = -2.0
BFQ = 256          # queries [0, BFQ) take the bf16 path


def xr(ap, pattern, **kw):
    return ap.rearrange(pattern, **kw)


def build_bass():
    nc = bacc.Bacc()
    xtbf = nc.declare_dram_parameter("xtbf", [DIM, BFQ], BF16, isOutput=False)
    xt8 = nc.declare_dram_parameter("xt8", [512, 2 * T], F8, isOutput=False)
    wqbf = nc.declare_dram_parameter("wqbf", [DIM, 3 * HG], BF16, isOutput=False)
    wq8 = nc.declare_dram_parameter("wq8", [512, 2 * 3 * HG], F8, isOutput=False)
    woutt = nc.declare_dram_parameter("woutt", [HG, DIM], BF16, isOutput=False)
    trik8 = nc.declare_dram_parameter("trik8", [128, 128], F8, isOutput=False)
    trikb = nc.declare_dram_parameter("trikb", [128, 128], BF16, isOutput=False)
    yt = nc.declare_dram_parameter("yt", [DIM, T], BF16, isOutput=True)

    with tile.TileContext(nc) as tc:
        _emit(nc, tc, xtbf, xt8, wqbf, wq8, woutt, trik8, trikb, yt)
    nc.finalize()
    return nc


def _emit(nc, tc, xtbf, xt8, wqbf, wq8, woutt, trik8, trikb, yt):
    ctx = contextlib.ExitStack()
    with ctx:
        singles = ctx.enter_context(tc.tile_pool(name="singles", bufs=1))
        xpool = ctx.enter_context(tc.tile_pool(name="xpool", bufs=2))
        epool = ctx.enter_context(tc.tile_pool(name="epool", bufs=3))
        apool = ctx.enter_context(tc.tile_pool(name="apool", bufs=3))
        spool = ctx.enter_context(tc.tile_pool(name="spool", bufs=1))
        dpool = ctx.enter_context(tc.tile_pool(name="dpool", bufs=2, space="DRAM"))
        # PSUM budget (8 banks of 2KB/partition):
        #   psq  [128,1024] bufs=2 -> 4 banks (score quads, double-buffered)
        #   psot [66,512]   bufs=3 -> 3 banks (per-head PV accumulators)
        #   psmm [128,512]  bufs=1 -> 1 bank (stage 1 + stage 3 groups)
        psq = ctx.enter_context(tc.tile_pool(name="psq", bufs=2, space="PSUM"))
        psot = ctx.enter_context(tc.tile_pool(name="psot", bufs=3, space="PSUM"))
        psmm = ctx.enter_context(tc.tile_pool(name="psmm", bufs=1, space="PSUM"))

        # ---- persistent SBUF tensors; DMA order = dependency order ----
        # interleave weight/x DMAs so the first stage-1 group can start after
        # only one (w, x) tile pair has landed
        wqb = []
        xts0 = []
        for k in range(8):
            w = singles.tile([128, 3 * HG], BF16, name=f"wqb{k}")
            nc.sync.dma_start(out=w[:, 0:HG], in_=wqbf[k * 128:(k + 1) * 128, 0:HG])
            wqb.append(w)
            xtile = xpool.tile([128, BFQ], BF16, tag=f"xb{k}", name=f"xb{k}")
            nc.sync.dma_start(out=xtile, in_=xtbf[k * 128:(k + 1) * 128, :])
            xts0.append(xtile)
        for k in range(8):
            nc.sync.dma_start(out=wqb[k][:, HG:3 * HG],
                              in_=wqbf[k * 128:(k + 1) * 128, HG:3 * HG])
        wq8s = []
        for k in range(4):
            w = singles.tile([128, 2, 3 * HG], F8, name=f"wq8{k}")
            nc.sync.dma_start(out=xr(w, "p s c -> p (s c)"),
                              in_=wq8[k * 128:(k + 1) * 128, :])
            wq8s.append(w)

        def x8_tiles(c):
            cs = slice(c * 512, (c + 1) * 512)
            x8s = []
            for k in range(4):
                t8 = xpool.tile([128, 2, 512], F8, tag=f"x8_{k}",
                                name=f"x8_{c}_{k}")
                nc.sync.dma_start(
                    out=t8,
                    in_=xr(xt8[k * 128:(k + 1) * 128, :],
                           "p (s t) -> p s t", s=2)[:, :, cs])
                x8s.append(t8)
            return x8s

        x8s0 = x8_tiles(0)

        wo = []
        for k in range(4):
            w = singles.tile([128, DIM], BF16, name=f"wo{k}")
            nc.sync.dma_start(out=w, in_=woutt[k * 128:(k + 1) * 128, :])
            wo.append(w)
        tk8 = singles.tile([128, 128], F8, name="tk8")
        nc.sync.dma_start(out=tk8, in_=trik8[:, :])
        tkb = singles.tile([128, 128], BF16, name="tkb")
        nc.sync.dma_start(out=tkb, in_=trikb[:, :])
        biasap = singles.tile([128, 1], F32, name="expbias")
        nc.gpsimd.memset(biasap, EXP_BIAS)

        # bf16 q/k for queries/keys [0, BFQ): 4 tiles [128, BFQ]
        qtb = [singles.tile([128, BFQ], BF16, name=f"qtb{m}") for m in range(4)]
        ktb = [singles.tile([128, BFQ], BF16, name=f"ktb{m}") for m in range(4)]
        # bf16 v for keys [0, BFQ): 2 token-tiles [128, 8, 65]
        vtb = [singles.tile([128, HEADS_PER_CORE, 65], BF16, name=f"vtb{t}")
               for t in range(BFQ // 128)]
        for t in range(BFQ // 128):
            nc.gpsimd.memset(vtb[t][:, :, 64:65], 1.0)
        # fp8 q for tokens [BFQ, T): [4h x 32d, 2 dim-half, T - BFQ]
        qt8 = [singles.tile([128, 2, T - BFQ], F8, name=f"qt8{g}")
               for g in range(2)]
        # fp8 k (all tokens): [128, 2, 2048]
        kt8 = [singles.tile([128, 2, T], F8, name=f"kt8{g}") for g in range(2)]
        # fp8 v: 8 ktile-pair tiles; inner stride 80 (dual-fp8 LDW needs
        # the sub-ktile step to be a multiple of 16), 66 cols used
        vt8 = [singles.tile([128, HEADS_PER_CORE, 2, 80], F8, name=f"vt8{tp}")
               for tp in range(8)]
        for tp in range(8):
            nc.gpsimd.memset(vt8[tp][:, :, :, 64:65], 1.0)
            nc.gpsimd.memset(vt8[tp][:, :, :, 65:66], 0.0)

        # During stage1_ch0 the attention quad banks are still free: rotate
        # its psum groups over 3 banks so each group's psum->SBUF copy
        # overlaps the next groups' matmuls (psmm alone serializes them).
        s1_ctr = [0]

        def s1_ps(name):
            i = s1_ctr[0]
            s1_ctr[0] += 1
            if i % 3 == 0:
                return psmm.tile([128, 512], F32, tag="qkv", name=name)
            return psq.tile([128, 1024], F32, tag="quad", name=name)[:, 0:512]

        def stage1_ch0():
            """Chunk 0: bf16 QKV over [0,BFQ) + fp8 q/k/v as needed."""
            # bf16 q, k over tokens [0, BFQ)
            for which, dst in ((0, qtb), (1, ktb)):
                for m in range(4):
                    ps = s1_ps(f"pb{which}{m}")[:, 0:BFQ]
                    for k in range(8):
                        nc.tensor.matmul(
                            ps,
                            lhsT=wqb[k][:, which * HG + m * 128:
                                        which * HG + (m + 1) * 128],
                            rhs=xts0[k],
                            start=(k == 0), stop=(k == 7))
                    nc.vector.tensor_copy(dst[m], ps)
            # bf16 v over token tiles [0, BFQ) + fp8 casts of the same psum
            for i in range(BFQ // 128):
                ps = s1_ps(f"pbv{i}")
                for k in range(8):
                    nc.tensor.matmul(
                        ps,
                        lhsT=xts0[k][:, i * 128:(i + 1) * 128],
                        rhs=wqb[k][:, 2 * HG:3 * HG],
                        start=(k == 0), stop=(k == 7))
                ps3 = xr(ps, "p (h d) -> p h d", h=HEADS_PER_CORE)
                nc.vector.tensor_copy(vtb[i][:, :, 0:64], ps3)
                nc.vector.tensor_copy(vt8[i // 2][:, :, i % 2, 0:64], ps3)
            # fp8 k for chunk 0 (all 512 keys)
            for m in range(4):
                ps = s1_ps(f"p8k{m}")
                for k in range(4):
                    nc.tensor.matmul(
                        ps,
                        lhsT=wq8s[k][:, :, HG + m * 128:HG + (m + 1) * 128],
                        rhs=x8s0[k],
                        start=(k == 0), stop=(k == 3), perf_mode=DR)
                nc.vector.tensor_copy(kt8[m // 2][:, m % 2, 0:512], ps)
            # fp8 q for tokens [BFQ, 512)
            for m in range(4):
                ps = s1_ps(f"p8q{m}")[:, 0:512 - BFQ]
                for k in range(4):
                    nc.tensor.matmul(
                        ps,
                        lhsT=wq8s[k][:, :, m * 128:(m + 1) * 128],
                        rhs=x8s0[k][:, :, BFQ:512],
                        start=(k == 0), stop=(k == 3), perf_mode=DR)
                nc.vector.tensor_copy(qt8[m // 2][:, m % 2, 0:512 - BFQ], ps)
            # fp8 v for token tiles [BFQ, 512)
            for i in range(BFQ // 128, 4):
                ps = s1_ps(f"p8v{i}")
                for k in range(4):
                    nc.tensor.matmul(
                        ps,
                        lhsT=x8s0[k][:, :, i * 128:(i + 1) * 128],
                        rhs=wq8s[k][:, :, 2 * HG:3 * HG],
                        start=(k == 0), stop=(k == 3), perf_mode=DR)
                nc.vector.tensor_copy(
                    vt8[i // 2][:, :, i % 2, 0:64],
                    xr(ps, "p (h d) -> p h d", h=HEADS_PER_CORE))

        def stage1_fp8_fillers(c):
            """fp8 QKV for chunk c>=1, as filler closures (4 matmuls each)."""
            cs = slice(c * 512, (c + 1) * 512)
            qcs = slice(c * 512 - BFQ, (c + 1) * 512 - BFQ)
            x8s = x8_tiles(c)
            fillers = []

            def qk_group(which, m):
                def go():
                    ps = psmm.tile([128, 512], F32, tag="qkv",
                                   name=f"p8{which}_{c}_{m}")
                    for k in range(4):
                        nc.tensor.matmul(
                            ps,
                            lhsT=wq8s[k][:, :, which * HG + m * 128:
                                         which * HG + (m + 1) * 128],
                            rhs=x8s[k],
                            start=(k == 0), stop=(k == 3), perf_mode=DR)
                    dst = (qt8 if which == 0 else kt8)[m // 2]
                    nc.vector.tensor_copy(
                        dst[:, m % 2, qcs if which == 0 else cs], ps)
                return go

            def v_group(i):
                def go():
                    t = 4 * c + i
                    ps = psmm.tile([128, 512], F32, tag="qkv", name=f"p8V{c}{i}")
                    for k in range(4):
                        nc.tensor.matmul(
                            ps,
                            lhsT=x8s[k][:, :, i * 128:(i + 1) * 128],
                            rhs=wq8s[k][:, :, 2 * HG:3 * HG],
                            start=(k == 0), stop=(k == 3), perf_mode=DR)
                    nc.vector.tensor_copy(
                        vt8[t // 2][:, :, t % 2, 0:64],
                        xr(ps, "p (h d) -> p h d", h=HEADS_PER_CORE))
                return go

            for m in range(4):
                fillers.append(qk_group(1, m))
            for i in range(4):
                fillers.append(v_group(i))
            for m in range(4):
                fillers.append(qk_group(0, m))
            return fillers

        # ---------------- attention building blocks ----------------
        # A quad holds scores for (head h, key-tiles 2tp and 2tp+1) over a
        # query window of width U: quad cols [sub*U, sub*U + U).
        # "live0" = first live query col (relative to the window) of sub0;
        # sub1's live range starts 128 later.  Window col 0 = query qlo.

        def scores_f8(h, quad, et, tp, qlo, U, diag):
            g, hi = h // 4, 32 * (h % 4)
            qg = qt8[g]
            qoff = qlo - BFQ
            for sub in range(2):
                t = 2 * tp + sub
                lhsT = kt8[g][hi:hi + 32, :, t * 128:(t + 1) * 128]
                base = sub * U
                if not diag:
                    nc.tensor.matmul(
                        quad[:, base:base + U],
                        lhsT=lhsT, rhs=qg[hi:hi + 32, :, qoff:qoff + U],
                        start=True, stop=True, tile_position=(hi, 0),
                        perf_mode=DR)
                    continue
                lo = sub * 128          # dead cols [0, lo) for sub1
                if sub == 1:
                    nc.gpsimd.memset(et[:, U:U + 128], 0.0)
                nc.tensor.matmul(
                    quad[:, base + lo:base + U],
                    lhsT=lhsT, rhs=qg[hi:hi + 32, :, qoff + lo:qoff + U],
                    start=True, stop=True, tile_position=(hi, 0), perf_mode=DR)

        def scores_bf(h, quad, tp, U, diag):
            m, ho = h // 2, 64 * (h % 2)
            for sub in range(2):
                t = 2 * tp + sub
                lhsT = ktb[m][ho:ho + 64, t * 128:(t + 1) * 128]
                base = sub * U
                lo = sub * 128 if diag else 0
                nc.tensor.matmul(
                    quad[:, base + lo:base + U],
                    lhsT=lhsT, rhs=qtb[m][ho:ho + 64, lo:U],
                    start=True, stop=True, tile_position=(ho, 0))

        def exp_quad(et, quad, U, diag, split, mask):
            if diag and split:
                nc.scalar.activation(et[:, 0:U], quad[:, 0:U], EXPF,
                                     scale=float(SCALE), bias=biasap)
                nc.scalar.activation(et[:, U + 128:2 * U], quad[:, U + 128:2 * U],
                                     EXPF, scale=float(SCALE), bias=biasap)
                # boundary triangles: keep-mask on the (otherwise idle) Pool
                nc.gpsimd.tensor_mul(et[:, 0:128], et[:, 0:128], mask)
                nc.gpsimd.tensor_mul(et[:, U + 128:U + 256],
                                     et[:, U + 128:U + 256], mask)
            else:
                nc.scalar.activation(et[:, 0:2 * U], quad[:, 0:2 * U], EXPF,
                                     scale=float(SCALE), bias=biasap)

        def attention(c, fillers):
            cs = slice(c * 512, (c + 1) * 512)
            aot = [apool.tile([128, 512], BF16, tag=f"aot{k}", name=f"aot{c}_{k}")
                   for k in range(4)]
            npairs = 2 * (c + 1)
            for h in range(8):
                ot = psot.tile([66, 512], F32, tag="ot", name=f"ot{c}_{h}")
                if c == 0:
                    _attn_head_ch0(h, ot)
                else:
                    for tp in range(npairs):
                        diag = tp >= npairs - 2
                        U = 256 if (diag and tp == npairs - 1) else 512
                        u0 = 512 - U
                        quad = psq.tile([128, 1024], F32, tag="quad",
                                        name=f"q{c}_{h}_{tp}")
                        et = epool.tile([128, 1024], F8, tag="e8",
                                        name=f"e{c}_{h}_{tp}")
                        scores_f8(h, quad, et, tp, c * 512 + u0, U, diag)
                        exp_quad(et, quad, U, diag, split=True, mask=tk8)
                        nc.tensor.matmul(
                            ot[:, u0:u0 + U],
                            lhsT=vt8[tp][:, h, :, 0:66],
                            rhs=xr(et[:, 0:2 * U], "p (s u) -> p s u", s=2),
                            start=(tp == 0), stop=(tp == npairs - 1),
                            perf_mode=DR)
                # normalize: row 64 of ot is the denominator
                den = spool.tile([1, 512], F32, tag="den", name=f"dn{c}_{h}")
                nc.vector.tensor_copy(den, ot[64:65, :])
                recf = spool.tile([1, 512], F32, tag="recf", name=f"rf{c}_{h}")
                nc.vector.reciprocal_approx_fast(recf, den)
                recb = spool.tile([1, 512], BF16, tag="recb", name=f"rb{c}_{h}")
                nc.vector.tensor_copy(recb, recf)
                dr = dpool.tile([1, 512], BF16, tag="dr", name=f"dr{c}_{h}")
                nc.sync.dma_start(out=dr, in_=recb)
                bcs = spool.tile([64, 512], BF16, tag="bcs", bufs=2,
                                 name=f"bs{c}_{h}")
                nc.sync.dma_start(out=bcs, in_=dr.to_broadcast((64, 512)))
                nc.vector.tensor_mul(
                    aot[h // 2][(h % 2) * 64:(h % 2) * 64 + 64, :],
                    ot[0:64, :], bcs)
                # pump fillers so the PE queue has QKV/out-proj work while
                # the exp + normalize chains of this head drain
                for _ in range(2):
                    if fillers:
                        fillers.popleft()()
            while fillers:
                fillers.popleft()()
            return aot

        def _attn_head_ch0(h, ot):
            # block A: queries [0, BFQ) in bf16 over key-tiles 0..BFQ/128-1
            UA = BFQ // 2  # window width per sub (2 ktiles span BFQ keys)
            quad = psq.tile([128, 1024], F32, tag="quad", name=f"qA_{h}")
            etA = epool.tile([128, 1024], BF16, tag="eb", name=f"eA_{h}")
            scores_bf(h, quad, 0, BFQ, diag=True)
            exp_quad(etA, quad, BFQ, diag=True, split=True, mask=tkb)
            nc.tensor.matmul(
                ot[0:65, 0:BFQ], lhsT=vtb[0][:, h, :], rhs=etA[:, 0:BFQ],
                start=True, stop=False, skip_group_check=True)
            nc.tensor.matmul(
                ot[0:65, 128:BFQ], lhsT=vtb[1][:, h, :],
                rhs=etA[:, BFQ + 128:2 * BFQ],
                start=False, stop=False, skip_group_check=True)
            # block B: queries [BFQ, 512) in fp8 over all 4 chunk-0 ktiles
            UB = 512 - BFQ
            for tp in range(2):
                diag = tp == 1  # ktiles 2,3 overlap the B query range
                quad = psq.tile([128, 1024], F32, tag="quad", name=f"qB_{h}_{tp}")
                et = epool.tile([128, 1024], F8, tag="e8", name=f"eB_{h}_{tp}")
                scores_f8(h, quad, et, tp, BFQ, UB, diag)
                exp_quad(et, quad, UB, diag, split=True, mask=tk8)
                nc.tensor.matmul(
                    ot[:, BFQ:512],
                    lhsT=vt8[tp][:, h, :, 0:66],
                    rhs=xr(et[:, 0:2 * UB], "p (s u) -> p s u", s=2),
                    start=False, stop=(tp == 1), skip_group_check=True,
                    perf_mode=DR)

        def stage3_fillers(c, aot):
            cs = slice(c * 512, (c + 1) * 512)
            fillers = []

            def out_group(od):
                def go():
                    # for the final chunk the score-quad banks are free:
                    # alternate pools so consecutive groups double-buffer
                    if c == NCH - 1 and od % 2 == 1:
                        ps = psq.tile([128, 1024], F32, tag="quad",
                                      name=f"py{c}_{od}")[:, 0:512]
                    else:
                        ps = psmm.tile([128, 512], F32, tag="qkv",
                                       name=f"py{c}_{od}")
                    for k in range(4):
                        nc.tensor.matmul(
                            ps,
                            lhsT=wo[k][:, od * 128:(od + 1) * 128],
                            rhs=aot[k],
                            start=(k == 0), stop=(k == 3))
                    ys = spool.tile([128, 512], BF16, tag="ys", bufs=2,
                                    name=f"ys{c}_{od}")
                    if c == NCH - 1 and od >= 5:
                        nc.scalar.copy(ys, ps)   # ACT is idle at the tail
                    else:
                        nc.vector.tensor_copy(ys, ps)
                    nc.sync.dma_start(out=yt[od * 128:(od + 1) * 128, cs], in_=ys)
                return go

            for od in range(8):
                fillers.append(out_group(od))
            return fillers

        # ---- schedule ----
        # stage-3 fillers are deferred a chunk so the later (bigger)
        # attention chunks keep the PE fed while exp chains drain
        stage1_ch0()
        s3 = {}
        for c in range(NCH):
            fl = deque()
            if c + 1 < NCH:
                fl.extend(stage1_fp8_fillers(c + 1))
            if c == 1:
                fl.extend(s3[0])
            elif c == 2:
                fl.extend(s3[1][:4])
            elif c == 3:
                fl.extend(s3[1][4:])
                fl.extend(s3[2])
            aot = attention(c, fl)
            s3[c] = stage3_fillers(c, aot)
        for f in s3[NCH - 1]:
            f()


_NC_CACHE = None


def _get_nc():
    global _NC_CACHE
    if _NC_CACHE is None:
        _NC_CACHE = build_bass()
    return _NC_CACHE


def _keep_pattern():
    k = np.arange(128)[:, None]
    q = np.arange(128)[None, :]
    return (q >= k).astype(np.float32)        # 1 = attended


def make_in_maps(x, w_qkv, w_out):
    x = np.asarray(x, dtype=np.float32)
    w_qkv = np.asarray(w_qkv, dtype=np.float32)
    w_out = np.asarray(w_out, dtype=np.float32)
    pat = _keep_pattern()
    f8 = ml_dtypes.float8_e4m3
    bf = ml_dtypes.bfloat16

    in_maps = []
    for core in range(N_CORES):
        b, g = core // 2, core % 2
        gs = slice(g * HG, (g + 1) * HG)
        wsel = np.concatenate(
            [w_qkv[0 * INNER:][gs], w_qkv[1 * INNER:][gs], w_qkv[2 * INNER:][gs]],
            axis=0)                               # [1536, 1024] bf16 order
        # fp8 weight column order: q/k in (grp, dim-half) blocks of 4h x 32d
        cols = np.empty(3 * HG, np.int64)
        j = np.arange(HG)
        m, r = j // 128, j % 128
        hh = g * 8 + (m // 2) * 4 + r // 32
        d = (m % 2) * 32 + (r % 32)
        cols[0:HG] = hh * 64 + d
        cols[HG:2 * HG] = INNER + hh * 64 + d
        cols[2 * HG:] = 2 * INNER + (g * 8 + j // 64) * 64 + (j % 64)
        wsel8 = w_qkv[cols, :]                    # [1536, 1024]
        # wq8 dram [512, 3072]: row 128k+p, col i*1536+j = wsel8[j, 256k+128i+p]
        wq8d = wsel8.T.reshape(4, 2, 128, 3 * HG).transpose(0, 2, 1, 3)
        wq8d = np.ascontiguousarray(wq8d.reshape(512, 2 * 3 * HG))
        # xt8 dram [512, 4096]: row 128k+p, col i*2048+t = x[b][t, 256k+128i+p]
        xt8d = x[b].T.reshape(4, 2, 128, T).transpose(0, 2, 1, 3)
        xt8d = np.ascontiguousarray(xt8d.reshape(512, 2 * T))
        in_maps.append({
            "xtbf": np.ascontiguousarray(x[b][0:BFQ].T).astype(bf),
            "xt8": xt8d.astype(f8),
            "wqbf": np.ascontiguousarray(wsel.T).astype(bf),
            "wq8": wq8d.astype(f8),
            "woutt": np.ascontiguousarray(w_out[:, gs].T).astype(bf),
            "trik8": pat.astype(f8),
            "trikb": pat.astype(bf),
        })
    return in_maps


def kernel(x, mask, w_qkv, w_out, **_):
    nc = _get_nc()
    in_maps = make_in_maps(x, w_qkv, w_out)
    res = run_bass_kernel_spmd(nc, in_maps, core_ids=list(range(N_CORES)))
    y = np.zeros((B, T, DIM), dtype=np.float32)
    for c in range(N_CORES):
        y[c // 2] += res.results[c]["yt"].astype(np.float32).T
    return y


# revision 25
# speedup vs baseline: 1.0882x; 1.0048x over previous
"""Causal self-attention Trainium2 Bass kernel (fp8 DoubleRow version).

Problem: B=4, T=2048, DIM=1024, H=16 heads, head_dim=64 (fp32).
  qkv = x @ w_qkv.T ; per-head causal softmax(q k^T / 8) v ; out @ w_out.T

Sharding (8 cores): core c -> (batch b = c//2, head-group g = c%2 of 8 heads).
Each core computes a partial output y_partial = attn_out_g @ w_out[:, g]^T
for its batch; host sums the two head-group partials per batch.

Precision scheme (validated vs fp32 reference, sim rel max err 0.0115 < 2e-2):
  - QKV projection in fp8e4 (e4m3) with MatmulPerfMode.DoubleRow: each
    matmul contracts 2 k-subtiles of 128 per pass.
  - Scores in fp8 DoubleRow: q/k stored [32 part, 2 dim-half, tokens]
    per 4-head group (weight columns pre-ordered on host), K = 32 x 2.
  - PV in fp8 DoubleRow over key-tile pairs: lhsT = v [128, 2, 66]
    (col 64 of ones emits the softmax denominator row; col 65 zero pad;
    sub-stride 80 because dual-fp8 LDWEIGHTS requires step % 16 == 0).
  - exp computes exp(s/8 - 2): the -2 bias keeps e < 240 (fp8 max);
    it cancels in the normalization.
  - Queries 0:256 (few attended keys -> fp8 noise doesn't average out)
    use a bf16 path: bf16 QKV over tokens 0:256 + bf16 scores/PV.
  - Output projection in bf16.

Causal masking: work above the diagonal is skipped.  Diagonal quads
slice scores/exp to the live query range per key-subtile; the
fully-dead leading block of each odd subtile is never computed (its e
range is memset to 0 for the fp8 PV; the bf16 PV just slices it out);
the 128x128 boundary triangle of each subtile is zeroed by a keep-mask
multiply on the (otherwise idle) GpSimd/Pool engine after exp.

The PE work is interleaved: QKV projection ("stage 1") of chunk c+1
and output projection ("stage 3") of earlier chunks are pumped as
fillers between attention heads, so the tensor engine stays busy while
exp/normalize chains drain (idle PE gaps also drop the HAM clock gate
to half rate, so continuous occupancy matters twice).
"""

import contextlib
from collections import deque

import numpy as np
import ml_dtypes

import concourse.mybir as mybir
import concourse.tile as tile
from concourse import bacc
from concourse.bass_utils import run_bass_kernel_spmd

B, T, DIM = 4, 2048, 1024
NUM_HEADS, HEAD_DIM = 16, 64
INNER = NUM_HEADS * HEAD_DIM
SCALE = HEAD_DIM ** -0.5

N_CORES = 8
HEADS_PER_CORE = 8
HG = HEADS_PER_CORE * HEAD_DIM  # 512 = inner slice per core
NCH = T // 512                  # 4 token chunks

F32 = mybir.dt.float32
BF16 = mybir.dt.bfloat16
F8 = mybir.dt.float8e4
DR = mybir.MatmulPerfMode.DoubleRow
EXPF = mybir.ActivationFunctionType.Exp

EXP_BIAS = -2.0
BFQ = 256          # queries [0, BFQ) take the bf16 path


def xr(ap, pattern, **kw):
    return ap.rearrange(pattern, **kw)


def build_bass():
    nc = bacc.Bacc()
    xtbf = nc.declare_dram_parameter("xtbf", [DIM, BFQ], BF16, isOutput=False)
    xt8 = nc.declare_dram_parameter("xt8", [512, 2 * T], F8, isOutput=False)
    wqbf = nc.declare_dram_parameter("wqbf", [DIM, 3 * HG], BF16, isOutput=False)
    wq8 = nc.declare_dram_parameter("wq8", [512, 2 * 3 * HG], F8, isOutput=False)
    woutt = nc.declare_dram_parameter("woutt", [HG, DIM], BF16, isOutput=False)
    trik8 = nc.declare_dram_parameter("trik8", [128, 128], F8, isOutput=False)
    trikb = nc.declare_dram_parameter("trikb", [128, 128], BF16, isOutput=False)
    yt = nc.declare_dram_parameter("yt", [DIM, T], BF16, isOutput=True)

    with tile.TileContext(nc) as tc:
        _emit(nc, tc, xtbf, xt8, wqbf, wq8, woutt, trik8, trikb, yt)
    nc.finalize()
    return nc


def _emit(nc, tc, xtbf, xt8, wqbf, wq8, woutt, trik8, trikb, yt):
    ctx = contextlib.ExitStack()
    with ctx:
        singles = ctx.enter_context(tc.tile_pool(name="singles", bufs=1))
        xpool = ctx.enter_context(tc.tile_pool(name="xpool", bufs=2))
        epool = ctx.enter_context(tc.tile_pool(name="epool", bufs=3))
        apool = ctx.enter_context(tc.tile_pool(name="apool", bufs=3))
        spool = ctx.enter_context(tc.tile_pool(name="spool", bufs=1))
        dpool = ctx.enter_context(tc.tile_pool(name="dpool", bufs=2, space="DRAM"))
        # PSUM budget (8 banks of 2KB/partition):
        #   psq  [128,1024] bufs=2 -> 4 banks (score quads, double-buffered)
        #   psot [66,512]   bufs=3 -> 3 banks (per-head PV accumulators)
        #   psmm [128,512]  bufs=1 -> 1 bank (stage 1 + stage 3 groups)
        psq = ctx.enter_context(tc.tile_pool(name="psq", bufs=2, space="PSUM"))
        psot = ctx.enter_context(tc.tile_pool(name="psot", bufs=3, space="PSUM"))
        psmm = ctx.enter_context(tc.tile_pool(name="psmm", bufs=1, space="PSUM"))

        # ---- persistent SBUF tensors; DMA order = dependency order ----
        # interleave weight/x DMAs so the first stage-1 group can start after
        # only one (w, x) tile pair has landed
        wqb = []
        xts0 = []
        for k in range(8):
            w = singles.tile([128, 3 * HG], BF16, name=f"wqb{k}")
            nc.sync.dma_start(out=w[:, 0:HG], in_=wqbf[k * 128:(k + 1) * 128, 0:HG])
            wqb.append(w)
            xtile = xpool.tile([128, BFQ], BF16, tag=f"xb{k}", name=f"xb{k}")
            nc.sync.dma_start(out=xtile, in_=xtbf[k * 128:(k + 1) * 128, :])
            xts0.append(xtile)
        for k in range(8):
            nc.sync.dma_start(out=wqb[k][:, HG:3 * HG],
                              in_=wqbf[k * 128:(k + 1) * 128, HG:3 * HG])
        wq8s = []
        for k in range(4):
            w = singles.tile([128, 2, 3 * HG], F8, name=f"wq8{k}")
            nc.sync.dma_start(out=xr(w, "p s c -> p (s c)"),
                              in_=wq8[k * 128:(k + 1) * 128, :])
            wq8s.append(w)

        def x8_tiles(c):
            cs = slice(c * 512, (c + 1) * 512)
            x8s = []
            for k in range(4):
                t8 = xpool.tile([128, 2, 512], F8, tag=f"x8_{k}",
                                name=f"x8_{c}_{k}")
                nc.sync.dma_start(
                    out=t8,
                    in_=xr(xt8[k * 128:(k + 1) * 128, :],
                           "p (s t) -> p s t", s=2)[:, :, cs])
                x8s.append(t8)
            return x8s

        x8s0 = x8_tiles(0)

        wo = []
        for k in range(4):
            w = singles.tile([128, DIM], BF16, name=f"wo{k}")
            nc.sync.dma_start(out=w, in_=woutt[k * 128:(k + 1) * 128, :])
            wo.append(w)
        tk8 = singles.tile([128, 128], F8, name="tk8")
        nc.sync.dma_start(out=tk8, in_=trik8[:, :])
        tkb = singles.tile([128, 128], BF16, name="tkb")
        nc.sync.dma_start(out=tkb, in_=trikb[:, :])
        biasap = singles.tile([128, 1], F32, name="expbias")
        nc.gpsimd.memset(biasap, EXP_BIAS)

        # bf16 q/k for queries/keys [0, BFQ): 4 tiles [128, BFQ]
        qtb = [singles.tile([128, BFQ], BF16, name=f"qtb{m}") for m in range(4)]
        ktb = [singles.tile([128, BFQ], BF16, name=f"ktb{m}") for m in range(4)]
        # bf16 v for keys [0, BFQ): 2 token-tiles [128, 8, 65]
        vtb = [singles.tile([128, HEADS_PER_CORE, 65], BF16, name=f"vtb{t}")
               for t in range(BFQ // 128)]
        for t in range(BFQ // 128):
            nc.gpsimd.memset(vtb[t][:, :, 64:65], 1.0)
        # fp8 q for tokens [BFQ, T): [4h x 32d, 2 dim-half, T - BFQ]
        qt8 = [singles.tile([128, 2, T - BFQ], F8, name=f"qt8{g}")
               for g in range(2)]
        # fp8 k (all tokens): [128, 2, 2048]
        kt8 = [singles.tile([128, 2, T], F8, name=f"kt8{g}") for g in range(2)]
        # fp8 v: 8 ktile-pair tiles; inner stride 80 (dual-fp8 LDW needs
        # the sub-ktile step to be a multiple of 16), 66 cols used
        vt8 = [singles.tile([128, HEADS_PER_CORE, 2, 80], F8, name=f"vt8{tp}")
               for tp in range(8)]
        for tp in range(8):
            nc.gpsimd.memset(vt8[tp][:, :, :, 64:65], 1.0)
            nc.gpsimd.memset(vt8[tp][:, :, :, 65:66], 0.0)

        # During stage1_ch0 the attention quad banks are still free: rotate
        # its psum groups over 3 banks so each group's psum->SBUF copy
        # overlaps the next groups' matmuls (psmm alone serializes them).
        s1_ctr = [0]

        def s1_ps(name):
            i = s1_ctr[0]
            s1_ctr[0] += 1
            if i % 3 == 0:
                return psmm.tile([128, 512], F32, tag="qkv", name=name)
            return psq.tile([128, 1024], F32, tag="quad", name=name)[:, 0:512]

        def stage1_ch0():
            """Chunk 0: bf16 QKV over [0,BFQ) + fp8 q/k/v as needed."""
            # bf16 q, k over tokens [0, BFQ)
            for which, dst in ((0, qtb), (1, ktb)):
                for m in range(4):
                    ps = s1_ps(f"pb{which}{m}")[:, 0:BFQ]
                    for k in range(8):
                        nc.tensor.matmul(
                            ps,
                            lhsT=wqb[k][:, which * HG + m * 128:
                                        which * HG + (m + 1) * 128],
                            rhs=xts0[k],
                            start=(k == 0), stop=(k == 7))
                    nc.vector.tensor_copy(dst[m], ps)
            # bf16 v over token tiles [0, BFQ) + fp8 casts of the same psum
            for i in range(BFQ // 128):
                ps = s1_ps(f"pbv{i}")
                for k in range(8):
                    nc.tensor.matmul(
                        ps,
                        lhsT=xts0[k][:, i * 128:(i + 1) * 128],
                        rhs=wqb[k][:, 2 * HG:3 * HG],
                        start=(k == 0), stop=(k == 7))
                ps3 = xr(ps, "p (h d) -> p h d", h=HEADS_PER_CORE)
                nc.vector.tensor_copy(vtb[i][:, :, 0:64], ps3)
                nc.vector.tensor_copy(vt8[i // 2][:, :, i % 2, 0:64], ps3)
            # fp8 k for chunk 0 (all 512 keys)
            for m in range(4):
                ps = s1_ps(f"p8k{m}")
                for k in range(4):
                    nc.tensor.matmul(
                        ps,
                        lhsT=wq8s[k][:, :, HG + m * 128:HG + (m + 1) * 128],
                        rhs=x8s0[k],
                        start=(k == 0), stop=(k == 3), perf_mode=DR)
                nc.vector.tensor_copy(kt8[m // 2][:, m % 2, 0:512], ps)
            # fp8 q for tokens [BFQ, 512)
            for m in range(4):
                ps = s1_ps(f"p8q{m}")[:, 0:512 - BFQ]
                for k in range(4):
                    nc.tensor.matmul(
                        ps,
                        lhsT=wq8s[k][:, :, m * 128:(m + 1) * 128],
                        rhs=x8s0[k][:, :, BFQ:512],
                        start=(k == 0), stop=(k == 3), perf_mode=DR)
                nc.vector.tensor_copy(qt8[m // 2][:, m % 2, 0:512 - BFQ], ps)
            # fp8 v for token tiles [BFQ, 512)
            for i in range(BFQ // 128, 4):
                ps = s1_ps(f"p8v{i}")
                for k in range(4):
                    nc.tensor.matmul(
                        ps,
                        lhsT=x8s0[k][:, :, i * 128:(i + 1) * 128],
                        rhs=wq8s[k][:, :, 2 * HG:3 * HG],
                        start=(k == 0), stop=(k == 3), perf_mode=DR)
                nc.vector.tensor_copy(
                    vt8[i // 2][:, :, i % 2, 0:64],
                    xr(ps, "p (h d) -> p h d", h=HEADS_PER_CORE))

        def stage1_fp8_fillers(c):
            """fp8 QKV for chunk c>=1, as filler closures (4 matmuls each)."""
            cs = slice(c * 512, (c + 1) * 512)
            qcs = slice(c * 512 - BFQ, (c + 1) * 512 - BFQ)
            x8s = x8_tiles(c)
            fillers = []

            def qk_group(which, m):
                def go():
                    ps = psmm.tile([128, 512], F32, tag="qkv",
                                   name=f"p8{which}_{c}_{m}")
                    for k in range(4):
                        nc.tensor.matmul(
                            ps,
                            lhsT=wq8s[k][:, :, which * HG + m * 128:
                                         which * HG + (m + 1) * 128],
                            rhs=x8s[k],
                            start=(k == 0), stop=(k == 3), perf_mode=DR)
                    dst = (qt8 if which == 0 else kt8)[m // 2]
                    nc.vector.tensor_copy(
                        dst[:, m % 2, qcs if which == 0 else cs], ps)
                return go

            def v_group(i):
                def go():
                    t = 4 * c + i
                    ps = psmm.tile([128, 512], F32, tag="qkv", name=f"p8V{c}{i}")
                    for k in range(4):
                        nc.tensor.matmul(
                            ps,
                            lhsT=x8s[k][:, :, i * 128:(i + 1) * 128],
                            rhs=wq8s[k][:, :, 2 * HG:3 * HG],
                            start=(k == 0), stop=(k == 3), perf_mode=DR)
                    nc.vector.tensor_copy(
                        vt8[t // 2][:, :, t % 2, 0:64],
                        xr(ps, "p (h d) -> p h d", h=HEADS_PER_CORE))
                return go

            for m in range(4):
                fillers.append(qk_group(1, m))
            for i in range(4):
                fillers.append(v_group(i))
            for m in range(4):
                fillers.append(qk_group(0, m))
            return fillers

        # ---------------- attention building blocks ----------------
        # A quad holds scores for (head h, key-tiles 2tp and 2tp+1) over a
        # query window of width U: quad cols [sub*U, sub*U + U).
        # "live0" = first live query col (relative to the window) of sub0;
        # sub1's live range starts 128 later.  Window col 0 = query qlo.

        def scores_f8(h, quad, et, tp, qlo, U, diag):
            g, hi = h // 4, 32 * (h % 4)
            qg = qt8[g]
            qoff = qlo - BFQ
            for sub in range(2):
                t = 2 * tp + sub
                lhsT = kt8[g][hi:hi + 32, :, t * 128:(t + 1) * 128]
                base = sub * U
                if not diag:
                    nc.tensor.matmul(
                        quad[:, base:base + U],
                        lhsT=lhsT, rhs=qg[hi:hi + 32, :, qoff:qoff + U],
                        start=True, stop=True, tile_position=(hi, 0),
                        perf_mode=DR)
                    continue
                lo = sub * 128          # dead cols [0, lo) for sub1
                if sub == 1:
                    nc.gpsimd.memset(et[:, U:U + 128], 0.0)
                nc.tensor.matmul(
                    quad[:, base + lo:base + U],
                    lhsT=lhsT, rhs=qg[hi:hi + 32, :, qoff + lo:qoff + U],
                    start=True, stop=True, tile_position=(hi, 0), perf_mode=DR)

        def scores_bf(h, quad, tp, U, diag):
            m, ho = h // 2, 64 * (h % 2)
            for sub in range(2):
                t = 2 * tp + sub
                lhsT = ktb[m][ho:ho + 64, t * 128:(t + 1) * 128]
                base = sub * U
                lo = sub * 128 if diag else 0
                nc.tensor.matmul(
                    quad[:, base + lo:base + U],
                    lhsT=lhsT, rhs=qtb[m][ho:ho + 64, lo:U],
                    start=True, stop=True, tile_position=(ho, 0))

        def exp_quad(et, quad, U, diag, split, mask):
            if diag and split:
                nc.scalar.activation(et[:, 0:U], quad[:, 0:U], EXPF,
                                     scale=float(SCALE), bias=biasap)
                nc.scalar.activation(et[:, U + 128:2 * U], quad[:, U + 128:2 * U],
                                     EXPF, scale=float(SCALE), bias=biasap)
                # boundary triangles: keep-mask on the (otherwise idle) Pool
                nc.gpsimd.tensor_mul(et[:, 0:128], et[:, 0:128], mask)
                nc.gpsimd.tensor_mul(et[:, U + 128:U + 256],
                                     et[:, U + 128:U + 256], mask)
            else:
                nc.scalar.activation(et[:, 0:2 * U], quad[:, 0:2 * U], EXPF,
                                     scale=float(SCALE), bias=biasap)

        def attention(c, fillers):
            cs = slice(c * 512, (c + 1) * 512)
            aot = [apool.tile([128, 512], BF16, tag=f"aot{k}", name=f"aot{c}_{k}")
                   for k in range(4)]
            npairs = 2 * (c + 1)
            for h in range(8):
                ot = psot.tile([66, 512], F32, tag="ot", name=f"ot{c}_{h}")
                if c == 0:
                    _attn_head_ch0(h, ot)
                else:
                    for tp in range(npairs):
                        diag = tp >= npairs - 2
                        U = 256 if (diag and tp == npairs - 1) else 512
                        u0 = 512 - U
                        quad = psq.tile([128, 1024], F32, tag="quad",
                                        name=f"q{c}_{h}_{tp}")
                        et = epool.tile([128, 1024], F8, tag="e8",
                                        name=f"e{c}_{h}_{tp}")
                        scores_f8(h, quad, et, tp, c * 512 + u0, U, diag)
                        exp_quad(et, quad, U, diag, split=True, mask=tk8)
                        nc.tensor.matmul(
                            ot[:, u0:u0 + U],
                            lhsT=vt8[tp][:, h, :, 0:66],
                            rhs=xr(et[:, 0:2 * U], "p (s u) -> p s u", s=2),
                            start=(tp == 0), stop=(tp == npairs - 1),
                            perf_mode=DR)
                # normalize: row 64 of ot is the denominator
                den = spool.tile([1, 512], F32, tag="den", name=f"dn{c}_{h}")
                nc.vector.tensor_copy(den, ot[64:65, :])
                recf = spool.tile([1, 512], F32, tag="recf", name=f"rf{c}_{h}")
                nc.vector.reciprocal_approx_fast(recf, den)
                recb = spool.tile([1, 512], BF16, tag="recb", name=f"rb{c}_{h}")
                nc.vector.tensor_copy(recb, recf)
                dr = dpool.tile([1, 512], BF16, tag="dr", name=f"dr{c}_{h}")
                nc.sync.dma_start(out=dr, in_=recb)
                bcs = spool.tile([64, 512], BF16, tag="bcs", bufs=2,
                                 name=f"bs{c}_{h}")
                nc.sync.dma_start(out=bcs, in_=dr.to_broadcast((64, 512)))
                nc.vector.tensor_mul(
                    aot[h // 2][(h % 2) * 64:(h % 2) * 64 + 64, :],
                    ot[0:64, :], bcs)
                # pump fillers so the PE queue has QKV/out-proj work while
                # the exp + normalize chains of this head drain; in the last
                # chunk hold half back so the post-loop drain covers the
                # final head's normalize latency before stage 3 can start
                for _ in range(2 if c < NCH - 1 else 1):
                    if fillers:
                        fillers.popleft()()
            while fillers:
                fillers.popleft()()
            return aot

        def _attn_head_ch0(h, ot):
            # block A: queries [0, BFQ) in bf16 over key-tiles 0..BFQ/128-1
            UA = BFQ // 2  # window width per sub (2 ktiles span BFQ keys)
            quad = psq.tile([128, 1024], F32, tag="quad", name=f"qA_{h}")
            etA = epool.tile([128, 1024], BF16, tag="eb", name=f"eA_{h}")
            scores_bf(h, quad, 0, BFQ, diag=True)
            exp_quad(etA, quad, BFQ, diag=True, split=True, mask=tkb)
            nc.tensor.matmul(
                ot[0:65, 0:BFQ], lhsT=vtb[0][:, h, :], rhs=etA[:, 0:BFQ],
                start=True, stop=False, skip_group_check=True)
            nc.tensor.matmul(
                ot[0:65, 128:BFQ], lhsT=vtb[1][:, h, :],
                rhs=etA[:, BFQ + 128:2 * BFQ],
                start=False, stop=False, skip_group_check=True)
            # block B: queries [BFQ, 512) in fp8 over all 4 chunk-0 ktiles
            UB = 512 - BFQ
            for tp in range(2):
                diag = tp == 1  # ktiles 2,3 overlap the B query range
                quad = psq.tile([128, 1024], F32, tag="quad", name=f"qB_{h}_{tp}")
                et = epool.tile([128, 1024], F8, tag="e8", name=f"eB_{h}_{tp}")
                scores_f8(h, quad, et, tp, BFQ, UB, diag)
                exp_quad(et, quad, UB, diag, split=True, mask=tk8)
                nc.tensor.matmul(
                    ot[:, BFQ:512],
                    lhsT=vt8[tp][:, h, :, 0:66],
                    rhs=xr(et[:, 0:2 * UB], "p (s u) -> p s u", s=2),
                    start=False, stop=(tp == 1), skip_group_check=True,
                    perf_mode=DR)

        def stage3_fillers(c, aot):
            cs = slice(c * 512, (c + 1) * 512)
            fillers = []

            def out_group(od):
                def go():
                    # for the final chunk the score-quad banks are free:
                    # alternate pools so consecutive groups double-buffer
                    if c == NCH - 1 and od % 2 == 1:
                        ps = psq.tile([128, 1024], F32, tag="quad",
                                      name=f"py{c}_{od}")[:, 0:512]
                    else:
                        ps = psmm.tile([128, 512], F32, tag="qkv",
                                       name=f"py{c}_{od}")
                    for k in range(4):
                        nc.tensor.matmul(
                            ps,
                            lhsT=wo[k][:, od * 128:(od + 1) * 128],
                            rhs=aot[k],
                            start=(k == 0), stop=(k == 3))
                    ys = spool.tile([128, 512], BF16, tag="ys", bufs=2,
                                    name=f"ys{c}_{od}")
                    if c == NCH - 1 and od in (5, 7):
                        nc.scalar.copy(ys, ps)   # ACT is idle at the tail
                    else:
                        nc.vector.tensor_copy(ys, ps)
                    nc.sync.dma_start(out=yt[od * 128:(od + 1) * 128, cs], in_=ys)
                return go

            for od in range(8):
                fillers.append(out_group(od))
            return fillers

        # ---- schedule ----
        # stage-3 fillers are deferred a chunk so the later (bigger)
        # attention chunks keep the PE fed while exp chains drain
        stage1_ch0()
        s3 = {}
        for c in range(NCH):
            fl = deque()
            if c + 1 < NCH:
                fl.extend(stage1_fp8_fillers(c + 1))
            if c == 1:
                fl.extend(s3[0])
            elif c == 2:
                fl.extend(s3[1][:4])
            elif c == 3:
                fl.extend(s3[1][4:])
                fl.extend(s3[2])
            aot = attention(c, fl)
            s3[c] = stage3_fillers(c, aot)
        for f in s3[NCH - 1]:
            f()


_NC_CACHE = None


def _get_nc():
    global _NC_CACHE
    if _NC_CACHE is None:
        _NC_CACHE = build_bass()
    return _NC_CACHE


def _keep_pattern():
    k = np.arange(128)[:, None]
    q = np.arange(128)[None, :]
    return (q >= k).astype(np.float32)        # 1 = attended


def make_in_maps(x, w_qkv, w_out):
    x = np.asarray(x, dtype=np.float32)
    w_qkv = np.asarray(w_qkv, dtype=np.float32)
    w_out = np.asarray(w_out, dtype=np.float32)
    pat = _keep_pattern()
    f8 = ml_dtypes.float8_e4m3
    bf = ml_dtypes.bfloat16

    in_maps = []
    for core in range(N_CORES):
        b, g = core // 2, core % 2
        gs = slice(g * HG, (g + 1) * HG)
        wsel = np.concatenate(
            [w_qkv[0 * INNER:][gs], w_qkv[1 * INNER:][gs], w_qkv[2 * INNER:][gs]],
            axis=0)                               # [1536, 1024] bf16 order
        # fp8 weight column order: q/k in (grp, dim-half) blocks of 4h x 32d
        cols = np.empty(3 * HG, np.int64)
        j = np.arange(HG)
        m, r = j // 128, j % 128
        hh = g * 8 + (m // 2) * 4 + r // 32
        d = (m % 2) * 32 + (r % 32)
        cols[0:HG] = hh * 64 + d
        cols[HG:2 * HG] = INNER + hh * 64 + d
        cols[2 * HG:] = 2 * INNER + (g * 8 + j // 64) * 64 + (j % 64)
        wsel8 = w_qkv[cols, :]                    # [1536, 1024]
        # wq8 dram [512, 3072]: row 128k+p, col i*1536+j = wsel8[j, 256k+128i+p]
        wq8d = wsel8.T.reshape(4, 2, 128, 3 * HG).transpose(0, 2, 1, 3)
        wq8d = np.ascontiguousarray(wq8d.reshape(512, 2 * 3 * HG))
        # xt8 dram [512, 4096]: row 128k+p, col i*2048+t = x[b][t, 256k+128i+p]
        xt8d = x[b].T.reshape(4, 2, 128, T).transpose(0, 2, 1, 3)
        xt8d = np.ascontiguousarray(xt8d.reshape(512, 2 * T))
        in_maps.append({
            "xtbf": np.ascontiguousarray(x[b][0:BFQ].T).astype(bf),
            "xt8": xt8d.astype(f8),
            "wqbf": np.ascontiguousarray(wsel.T).astype(bf),
            "wq8": wq8d.astype(f8),
            "woutt": np.ascontiguousarray(w_out[:, gs].T).astype(bf),
            "trik8": pat.astype(f8),
            "trikb": pat.astype(bf),
        })
    return in_maps


def kernel(x, mask, w_qkv, w_out, **_):
    nc = _get_nc()
    in_maps = make_in_maps(x, w_qkv, w_out)
    res = run_bass_kernel_spmd(nc, in_maps, core_ids=list(range(N_CORES)))
    y = np.zeros((B, T, DIM), dtype=np.float32)
    for c in range(N_CORES):
        y[c // 2] += res.results[c]["yt"].astype(np.float32).T
    return y
